# revision 14
# baseline (speedup 1.0000x reference)
"""Distributed MQA causal attention for TRN2 (8 NeuronCores).

Sharding: 8 cores = 2 (batch) x 4 (head-group tensor parallel).
Core c handles batch b=c//4, head group g=c%4 (8 heads, o-slice of 1024).
KV projection is replicated within a batch group.  After attention, the
per-core attn^T chunks are AllGather-ed (groups of 4) and each core computes
a 1024-wide column slice of the output projection.

All matmuls run in bf16 (f32 accumulation in PSUM).  The critical-path
operands (wq/wk/wv, x chunk 0) are cast f32->bf16 during the DMA load and
transposed on-chip with the (otherwise idle) TensorE so the first GEMM can
start after ~100us; the bulkier late operands (x chunks 1-3, wo) take the
DRAM bf16-bounce + DMA-transpose-xbar path, which overlaps compute.
"""

import numpy as np

import concourse.bass as bass
import concourse.mybir as mybir
import concourse.tile as tile
from concourse import bacc
from concourse.bass_utils import run_bass_kernel_spmd
from concourse.masks import make_identity

# Problem shape (hardcoded; kernel.py must be self-contained).
B, T, D = 2, 2048, 4096
H, HD = 32, 128
NCORES, TPG = 8, 4
HL = H // TPG            # 8 local heads per core
OL = HL * HD             # 1024 local q/o dims per core
P = 128
TC = 512                 # t-chunk width (moving-dim of the big GEMMs)
NTC = T // TC            # 4
ND = D // P              # 32 contraction tiles for D
NT = T // P              # 16 k-tiles
SCALE = float(1.0 / np.sqrt(HD))

BF16 = mybir.dt.bfloat16
F32 = mybir.dt.float32

_CACHE = {}
LAST_RESULT = None  # BassKernelResults of the most recent run (for test harness)


def build_nc():
    nc = bacc.Bacc(None, target_bir_lowering=False, num_devices=NCORES)

    x_ext = nc.declare_dram_parameter("x", [T, D], F32, isOutput=False)
    wq_ext = nc.declare_dram_parameter("wq", [OL, D], F32, isOutput=False)
    bq_ext = nc.declare_dram_parameter("bq", [OL], F32, isOutput=False)
    wk_ext = nc.declare_dram_parameter("wk", [HD, D], F32, isOutput=False)
    bk_ext = nc.declare_dram_parameter("bk", [HD], F32, isOutput=False)
    wv_ext = nc.declare_dram_parameter("wv", [HD, D], F32, isOutput=False)
    bv_ext = nc.declare_dram_parameter("bv", [HD], F32, isOutput=False)
    wo_ext = nc.declare_dram_parameter("wo", [OL, D], F32, isOutput=False)
    bo_ext = nc.declare_dram_parameter("bo", [OL], F32, isOutput=False)
    out_ext = nc.declare_dram_parameter("out", [T, OL], F32, isOutput=True)

    with tile.TileContext(nc) as tc:
        with (
            tc.tile_pool(name="consts", bufs=1) as consts,
            tc.tile_pool(name="wpool", bufs=1) as wpool,
            tc.tile_pool(name="wsmall", bufs=1) as wsmall,
            tc.tile_pool(name="slab", bufs=2) as slabp,
            tc.tile_pool(name="nat", bufs=2) as natp,
            tc.tile_pool(name="big", bufs=1) as bigp,
            tc.tile_pool(name="qtc", bufs=2) as qtcp,
            tc.tile_pool(name="atc", bufs=1) as atcp,
            tc.tile_pool(name="esb", bufs=2) as esbp,
            tc.tile_pool(name="tmp", bufs=3) as tmpp,
            tc.tile_pool(name="psum", bufs=1, space="PSUM") as psump,
            tc.tile_pool(name="dram", bufs=1, space="DRAM") as dram,
        ):
            # ---- Constants (tiny, first so gpsimd builds them before casts)
            ident = consts.tile([P, P], BF16)
            make_identity(nc, ident[:])
            # Causal 0/1 masks for the 4 diagonal-band offsets:
            # masks[i][x, y] = 1 if y - x - 128*i >= 0 else 0
            masks = []
            for i in range(4):
                m = consts.tile([P, TC], BF16, name=f"mask{i}")
                nc.gpsimd.memset(m[:], 1.0)
                nc.gpsimd.affine_select(
                    out=m[:],
                    in_=m[:],
                    pattern=[[1, TC]],
                    compare_op=mybir.AluOpType.is_ge,
                    fill=0.0,
                    base=-P * i,
                    channel_multiplier=-1,
                )
                masks.append(m)
            bq_sb = consts.tile([P, HL], F32)
            nc.sync.dma_start(bq_sb[:], bq_ext[:].rearrange("(o p) -> p o", p=P))
            bk_sb = consts.tile([P, 1], F32)
            nc.sync.dma_start(bk_sb[:], bk_ext[:].rearrange("(o p) -> p o", p=P))
            bv_sb = consts.tile([P, 1], F32)
            nc.sync.dma_start(bv_sb[:], bv_ext[:].rearrange("(o p) -> p o", p=P))
            bo_row = natp.tile([1, OL], BF16, tag="nat", name="bo_row")
            nc.gpsimd.dma_start(bo_row[:], bo_ext[None, :])
            bo_bc = consts.tile([P, OL], BF16)
            nc.gpsimd.partition_broadcast(bo_bc[:], bo_row[:])

            # Persistent attention operands.
            kT = bigp.tile([P, T], BF16, name="kT")               # [hd, t]
            vn = bigp.tile([P, NT, HD], BF16, name="vn")  # v natural [tk, kt, hd]
            ones_col = consts.tile([P, 1], BF16)
            nc.vector.memset(ones_col[:], 1.0)
            ones_row = consts.tile([1, P], BF16)
            nc.vector.memset(ones_row[:], 1.0)

            # ---- On-chip cast+transpose for critical-path operands ----------
            # src_ext [rows, D] f32 -> dstT[:, dt, col0 + 128*block] bf16
            def load_T_onchip(src_ext, rows, dstT, col0, what):
                for blk in range(rows // P):
                    nat = natp.tile([P, D], BF16, tag="nat", name=f"nat_{what}{blk}")
                    nc.gpsimd.dma_start(nat[:], src_ext[blk * P : (blk + 1) * P, :])
                    for g in range(0, ND, 4):
                        pst = psump.tile(
                            [P, 4 * P], BF16, tag="attA", bufs=2, name=f"ptr_{what}{blk}{g}"
                        )
                        for j in range(4):
                            nc.tensor.transpose(
                                pst[:, j * P : (j + 1) * P],
                                nat[:, (g + j) * P : (g + j + 1) * P],
                                ident[:],
                            )
                        nc.vector.tensor_copy(
                            dstT[:, g : g + 4, col0 + blk * P : col0 + (blk + 1) * P],
                            pst[:].rearrange("p (g t) -> p g t", g=4),
                        )

            wqT = wpool.tile([P, ND, OL], BF16, tag="bigw", name="wqT")
            load_T_onchip(wq_ext, OL, wqT, 0, "wq")
            wkT = wsmall.tile([P, ND, HD], BF16, name="wkT")
            load_T_onchip(wk_ext, HD, wkT, 0, "wk")
            wvT = wsmall.tile([P, ND, HD], BF16, name="wvT")
            load_T_onchip(wv_ext, HD, wvT, 0, "wv")

            # ---- DRAM bf16 scratch for the late operands (x1-3, wo) ---------
            x_bf = dram.tile([T, D], BF16)
            wo_bf = dram.tile([OL, D], BF16)

            def cast_to_scratch(src_ext, dst, r0, r1, what):
                step = P
                for i, r in enumerate(range(r0, r1, step)):
                    t_ = natp.tile([P, D], BF16, tag="nat", name=f"cs_{what}{i}")
                    nc.gpsimd.dma_start(t_[:], src_ext[r : r + step, :])
                    nc.sync.dma_start(dst[r : r + step, :], t_[:])

            # AllGather buffers, one per t-chunk (column-sliced attn^T).
            cc_in = [dram.tile([OL, TC], BF16, name=f"cc_in{c}") for c in range(NTC)]
            cc_g = [
                dram.tile([TPG * OL, TC], BF16, name=f"cc_g{c}") for c in range(NTC)
            ]

            # ------------- per t-chunk: QKV proj + attention ------------------
            for c in range(NTC):
                if c == 0:
                    # On-chip transpose: x rows 0..511 (critical path).
                    xT = slabp.tile([P, ND, TC], BF16, tag="slab", name="xT0")
                    for tb in range(TC // P):
                        natx = natp.tile([P, D], BF16, tag="nat", name=f"natx{tb}")
                        nc.gpsimd.dma_start(natx[:], x_ext[tb * P : (tb + 1) * P, :])
                        for g in range(0, ND, 4):
                            pst = psump.tile(
                                [P, 4 * P], BF16, tag="attA", bufs=2, name=f"ptrx{tb}{g}"
                            )
                            for j in range(4):
                                nc.tensor.transpose(
                                    pst[:, j * P : (j + 1) * P],
                                    natx[:, (g + j) * P : (g + j + 1) * P],
                                    ident[:],
                                )
                            nc.vector.tensor_copy(
                                xT[:, g : g + 4, tb * P : (tb + 1) * P],
                                pst[:].rearrange("p (g t) -> p g t", g=4),
                            )
                else:
                    xT = slabp.tile([P, ND, TC], BF16, tag="slab", name=f"xT{c}")
                    nc.sync.dma_start(
                        xT[:], x_bf[c * TC : (c + 1) * TC, :], transpose=True
                    )

                qTc = qtcp.tile([P, HL, TC], BF16, tag="qTc", name=f"qTc{c}")

                for ot in range(HL + 2):  # 0..7 = q heads, 8 = k, 9 = v
                    ps = psump.tile(
                        [P, TC], F32, tag="mm512", bufs=2, name=f"psqkv{c}_{ot}"
                    )
                    for dt in range(ND):
                        if ot < HL:
                            lhsT = wqT[:, dt, ot * P : (ot + 1) * P]
                        elif ot == HL:
                            lhsT = wkT[:, dt, :]
                        else:
                            lhsT = wvT[:, dt, :]
                        nc.tensor.matmul(
                            ps[:],
                            lhsT,
                            xT[:, dt, :],
                            start=(dt == 0),
                            stop=(dt == ND - 1),
                        )
                    if ot < HL:
                        nc.vector.tensor_scalar_add(
                            qTc[:, ot, :], ps[:], bq_sb[:, ot : ot + 1]
                        )
                    elif ot == HL:
                        nc.vector.tensor_scalar_add(
                            kT[:, c * TC : (c + 1) * TC], ps[:], bk_sb[:]
                        )
                    else:
                        vt = tmpp.tile([P, TC], BF16, tag="vt", bufs=1, name=f"vt{c}")
                        nc.vector.tensor_scalar_add(vt[:], ps[:], bv_sb[:])
                        pstv = psump.tile(
                            [P, 4 * P], BF16, tag="attA", bufs=2, name=f"pstv{c}"
                        )
                        for j in range(TC // P):
                            nc.tensor.transpose(
                                pstv[:, j * P : (j + 1) * P],
                                vt[:, j * P : (j + 1) * P],
                                ident[:],
                            )
                        nc.vector.tensor_copy(
                            vn[:, c * (TC // P) : (c + 1) * (TC // P), :],
                            pstv[:].rearrange("p (g t) -> p g t", g=4),
                        )

                # After chunk-0 compute is queued, enqueue the deferred casts:
                # x chunks 1-3 first (needed soonest), then wo.
                if c == 0:
                    cast_to_scratch(x_ext, x_bf, TC, T, "x")
                    cast_to_scratch(wo_ext, wo_bf, 0, OL, "wo")

                # Attention for all local heads on this q-chunk.
                attnT_c = atcp.tile([P, HL, TC], BF16, tag="atc", name=f"attnT{c}")
                nkt = (c + 1) * (TC // P)  # causal: k-tiles 0..nkt-1
                for h in range(HL):
                    att_acc = psump.tile(
                        [P, TC], F32, tag="attA", bufs=2, name=f"attA{c}_{h}"
                    )
                    sum_acc = psump.tile(
                        [1, TC], F32, tag="attS", bufs=2, name=f"attS{c}_{h}"
                    )
                    for kp in range(nkt // 2):
                        ps_s2 = psump.tile(
                            [P, 2, TC], F32, tag="mm512", bufs=2, name=f"pss{c}_{h}_{kp}"
                        )
                        for j in range(2):
                            kt = 2 * kp + j
                            nc.tensor.matmul(
                                ps_s2[:, j, :],
                                kT[:, kt * P : (kt + 1) * P],
                                qTc[:, h, :],
                                start=True,
                                stop=True,
                            )
                        es2 = esbp.tile(
                            [P, 2, TC], BF16, tag="esb", name=f"es{c}_{h}_{kp}"
                        )
                        nc.scalar.activation(
                            es2[:],
                            ps_s2[:],
                            mybir.ActivationFunctionType.Exp,
                            scale=SCALE,
                        )
                        for j in range(2):
                            kt = 2 * kp + j
                            if kt >= nkt - 4:
                                # Diagonal-band tile: zero weights where k > q.
                                nc.vector.tensor_tensor(
                                    es2[:, j, :],
                                    es2[:, j, :],
                                    masks[kt - (nkt - 4)][:],
                                    mybir.AluOpType.mult,
                                )
                            # attn^T[hd, tq] += v[kt].T @ es ; sums += 1.T @ es
                            nc.tensor.matmul(
                                att_acc[:],
                                vn[:, kt, :],
                                es2[:, j, :],
                                start=(kt == 0),
                                stop=(kt == nkt - 1),
                            )
                            nc.tensor.matmul(
                                sum_acc[:],
                                ones_col[:],
                                es2[:, j, :],
                                start=(kt == 0),
                                stop=(kt == nkt - 1),
                            )
                    sums_sb = esbp.tile([1, TC], BF16, tag="esb", name=f"ss{c}{h}")
                    nc.vector.tensor_copy(sums_sb[:], sum_acc[:])
                    bc_ps = psump.tile(
                        [P, TC], F32, tag="attS", bufs=2, name=f"bc{c}{h}"
                    )
                    nc.tensor.matmul(
                        bc_ps[:], ones_row[:], sums_sb[:], start=True, stop=True
                    )
                    recip_sb = tmpp.tile([P, TC], F32, tag="osb", bufs=1, name=f"rcs{c}{h}")
                    nc.vector.reciprocal(recip_sb[:], bc_ps[:])
                    nc.vector.tensor_tensor(
                        attnT_c[:, h, :], att_acc[:], recip_sb[:], mybir.AluOpType.mult
                    )

                # Ship this chunk's attn^T and AllGather it within the group.
                nc.sync.dma_start(
                    cc_in[c][:, :].rearrange("(h p) t -> p h t", p=P), attnT_c[:]
                )
                nc.gpsimd.collective_compute(
                    "AllGather",
                    mybir.AluOpType.bypass,
                    replica_groups=[[0, 1, 2, 3], [4, 5, 6, 7]],
                    ins=[cc_in[c][:, :].opt()],
                    outs=[cc_g[c][:, :].opt()],
                )

            # ---------------- Output projection -------------------------------
            woT = wpool.tile([P, ND, OL], BF16, tag="bigw", name="woT")
            nc.sync.dma_start(woT[:], wo_bf[:, :], transpose=True)

            for c in range(NTC):
                gT = slabp.tile([P, ND, TC], BF16, tag="slab", name=f"gT{c}")
                for ot in range(ND):
                    r, lh = divmod(ot, HL)
                    nc.sync.dma_start(
                        gT[:, ot, :],
                        cc_g[c][r * OL + lh * P : r * OL + (lh + 1) * P, :],
                    )
                for tt in range(TC // P):
                    for dc in range(OL // TC):
                        ps = psump.tile(
                            [P, TC], F32, tag="mm512", bufs=2, name=f"pso{c}_{tt}_{dc}"
                        )
                        for ot in range(ND):
                            nc.tensor.matmul(
                                ps[:],
                                gT[:, ot, tt * P : (tt + 1) * P],
                                woT[:, ot, dc * TC : (dc + 1) * TC],
                                start=(ot == 0),
                                stop=(ot == ND - 1),
                            )
                        osb = tmpp.tile([P, TC], F32, tag="osb", bufs=1, name=f"osb{c}{tt}{dc}")
                        nc.vector.tensor_tensor(
                            osb[:],
                            ps[:],
                            bo_bc[:, dc * TC : (dc + 1) * TC],
                            mybir.AluOpType.add,
                        )
                        nc.sync.dma_start(
                            out_ext[
                                c * TC + tt * P : c * TC + (tt + 1) * P,
                                dc * TC : (dc + 1) * TC,
                            ],
                            osb[:],
                        )

    nc.compile()
    return nc


def kernel(x, wq_w, wq_b, wk_w, wk_b, wv_w, wv_b, wo_w, wo_b):
    global LAST_RESULT
    if "nc" not in _CACHE:
        _CACHE["nc"] = build_nc()
    nc = _CACHE["nc"]

    f32 = np.float32
    x = np.asarray(x, f32)
    in_maps = []
    for c in range(NCORES):
        b, g = divmod(c, TPG)
        sl = slice(OL * g, OL * (g + 1))
        in_maps.append(
            {
                "x": np.ascontiguousarray(x[b]),
                "wq": np.ascontiguousarray(np.asarray(wq_w, f32)[sl]),
                "bq": np.ascontiguousarray(np.asarray(wq_b, f32)[sl]),
                "wk": np.ascontiguousarray(np.asarray(wk_w, f32)),
                "bk": np.ascontiguousarray(np.asarray(wk_b, f32)),
                "wv": np.ascontiguousarray(np.asarray(wv_w, f32)),
                "bv": np.ascontiguousarray(np.asarray(wv_b, f32)),
                "wo": np.ascontiguousarray(np.asarray(wo_w, f32)[sl]),
                "bo": np.ascontiguousarray(np.asarray(wo_b, f32)[sl]),
            }
        )

    res = run_bass_kernel_spmd(nc, in_maps, core_ids=list(range(NCORES)))
    LAST_RESULT = res

    out = np.empty((B, T, D), dtype=f32)
    for c in range(NCORES):
        b, g = divmod(c, TPG)
        out[b, :, OL * g : OL * (g + 1)] = res.results[c]["out"]
    return out


# revision 15
# speedup vs baseline: 1.1274x; 1.1274x over previous
"""Distributed MQA causal attention for TRN2 (8 NeuronCores).

Sharding: 8 cores = 2 (batch) x 4 (head-group tensor parallel).
Core c handles batch b=c//4, head group g=c%4 (8 heads, o-slice of 1024).
KV projection is replicated within a batch group.  After attention, the
per-core attn^T chunks are AllGather-ed (groups of 4) and each core computes
a 1024-wide column slice of the output projection.

All matmuls run in bf16 (f32 accumulation in PSUM).  The critical-path
operands (wq/wk/wv, x chunk 0) are cast f32->bf16 during the DMA load and
transposed on-chip with the (otherwise idle) TensorE so the first GEMM can
start after ~100us; the bulkier late operands (x chunks 1-3, wo) take the
DRAM bf16-bounce + DMA-transpose-xbar path, which overlaps compute.
"""

import numpy as np

import concourse.bass as bass
import concourse.mybir as mybir
import concourse.tile as tile
from concourse import bacc
from concourse.bass_utils import run_bass_kernel_spmd
from concourse.masks import make_identity

# Problem shape (hardcoded; kernel.py must be self-contained).
B, T, D = 2, 2048, 4096
H, HD = 32, 128
NCORES, TPG = 8, 4
HL = H // TPG            # 8 local heads per core
OL = HL * HD             # 1024 local q/o dims per core
P = 128
TC = 512                 # t-chunk width (moving-dim of the big GEMMs)
NTC = T // TC            # 4
ND = D // P              # 32 contraction tiles for D
NT = T // P              # 16 k-tiles
SCALE = float(1.0 / np.sqrt(HD))

BF16 = mybir.dt.bfloat16
F32 = mybir.dt.float32

_CACHE = {}
LAST_RESULT = None  # BassKernelResults of the most recent run (for test harness)


def build_nc():
    nc = bacc.Bacc(None, target_bir_lowering=False, num_devices=NCORES)

    x_ext = nc.declare_dram_parameter("x", [T, D], F32, isOutput=False)
    wq_ext = nc.declare_dram_parameter("wq", [OL, D], F32, isOutput=False)
    bq_ext = nc.declare_dram_parameter("bq", [OL], F32, isOutput=False)
    wk_ext = nc.declare_dram_parameter("wk", [HD, D], F32, isOutput=False)
    bk_ext = nc.declare_dram_parameter("bk", [HD], F32, isOutput=False)
    wv_ext = nc.declare_dram_parameter("wv", [HD, D], F32, isOutput=False)
    bv_ext = nc.declare_dram_parameter("bv", [HD], F32, isOutput=False)
    wo_ext = nc.declare_dram_parameter("wo", [OL, D], F32, isOutput=False)
    bo_ext = nc.declare_dram_parameter("bo", [OL], F32, isOutput=False)
    out_ext = nc.declare_dram_parameter("out", [T, OL], F32, isOutput=True)

    with tile.TileContext(nc) as tc:
        with (
            tc.tile_pool(name="consts", bufs=1) as consts,
            tc.tile_pool(name="wpool", bufs=1) as wpool,
            tc.tile_pool(name="wsmall", bufs=1) as wsmall,
            tc.tile_pool(name="slab", bufs=2) as slabp,
            tc.tile_pool(name="nat", bufs=2) as natp,
            tc.tile_pool(name="big", bufs=1) as bigp,
            tc.tile_pool(name="qtc", bufs=2) as qtcp,
            tc.tile_pool(name="atc", bufs=1) as atcp,
            tc.tile_pool(name="esb", bufs=2) as esbp,
            tc.tile_pool(name="tmp", bufs=3) as tmpp,
            tc.tile_pool(name="psum", bufs=1, space="PSUM") as psump,
            tc.tile_pool(name="dram", bufs=1, space="DRAM") as dram,
        ):
            # ---- Constants (tiny, first so gpsimd builds them before casts)
            ident = consts.tile([P, P], BF16)
            make_identity(nc, ident[:])
            # Causal 0/1 masks for the 4 diagonal-band offsets:
            # masks[i][x, y] = 1 if y - x - 128*i >= 0 else 0
            masks = []
            for i in range(4):
                m = consts.tile([P, TC], BF16, name=f"mask{i}")
                nc.gpsimd.memset(m[:], 1.0)
                nc.gpsimd.affine_select(
                    out=m[:],
                    in_=m[:],
                    pattern=[[1, TC]],
                    compare_op=mybir.AluOpType.is_ge,
                    fill=0.0,
                    base=-P * i,
                    channel_multiplier=-1,
                )
                masks.append(m)
            bq_sb = consts.tile([P, HL], F32)
            nc.sync.dma_start(bq_sb[:], bq_ext[:].rearrange("(o p) -> p o", p=P))
            bk_sb = consts.tile([P, 1], F32)
            nc.sync.dma_start(bk_sb[:], bk_ext[:].rearrange("(o p) -> p o", p=P))
            bv_sb = consts.tile([P, 1], F32)
            nc.sync.dma_start(bv_sb[:], bv_ext[:].rearrange("(o p) -> p o", p=P))
            bo_row = natp.tile([1, OL], BF16, tag="nat", name="bo_row")
            nc.gpsimd.dma_start(bo_row[:], bo_ext[None, :])
            bo_bc = consts.tile([P, OL], BF16)
            nc.gpsimd.partition_broadcast(bo_bc[:], bo_row[:])

            # Persistent attention operands.
            kT = bigp.tile([P, T], BF16, name="kT")               # [hd, t]
            vaug = bigp.tile([P, NT, HD + 1], BF16, name="vaug")  # [tk, kt, 129]
            nc.vector.memset(vaug[:, :, HD : HD + 1], 1.0)

            # ---- On-chip cast+transpose for critical-path operands ----------
            # src_ext [rows, D] f32 -> dstT[:, dt, col0 + 128*block] bf16
            def load_T_onchip(src_ext, rows, dstT, col0, what):
                for blk in range(rows // P):
                    nat = natp.tile([P, D], BF16, tag="nat", name=f"nat_{what}{blk}")
                    nc.gpsimd.dma_start(nat[:], src_ext[blk * P : (blk + 1) * P, :])
                    for g in range(0, ND, 4):
                        pst = psump.tile(
                            [P, 4 * P], BF16, tag="tr", bufs=2, name=f"ptr_{what}{blk}{g}"
                        )
                        for j in range(4):
                            nc.tensor.transpose(
                                pst[:, j * P : (j + 1) * P],
                                nat[:, (g + j) * P : (g + j + 1) * P],
                                ident[:],
                            )
                        nc.vector.tensor_copy(
                            dstT[:, g : g + 4, col0 + blk * P : col0 + (blk + 1) * P],
                            pst[:].rearrange("p (g t) -> p g t", g=4),
                        )

            wqT = wpool.tile([P, ND, OL], BF16, tag="bigw", name="wqT")
            load_T_onchip(wq_ext, OL, wqT, 0, "wq")
            wkT = wsmall.tile([P, ND, HD], BF16, name="wkT")
            load_T_onchip(wk_ext, HD, wkT, 0, "wk")
            wvT = wsmall.tile([P, ND, HD], BF16, name="wvT")
            load_T_onchip(wv_ext, HD, wvT, 0, "wv")

            # ---- DRAM bf16 scratch for the late operands (x1-3, wo) ---------
            x_bf = dram.tile([T, D], BF16)
            wo_bf = dram.tile([OL, D], BF16)

            def cast_to_scratch(src_ext, dst, r0, r1, what):
                step = P
                for i, r in enumerate(range(r0, r1, step)):
                    t_ = natp.tile([P, D], BF16, tag="nat", name=f"cs_{what}{i}")
                    nc.gpsimd.dma_start(t_[:], src_ext[r : r + step, :])
                    nc.sync.dma_start(dst[r : r + step, :], t_[:])

            # AllGather buffers, one per t-chunk (column-sliced attn^T).
            cc_in = [dram.tile([OL, TC], BF16, name=f"cc_in{c}") for c in range(NTC)]
            cc_g = [
                dram.tile([TPG * OL, TC], BF16, name=f"cc_g{c}") for c in range(NTC)
            ]

            # ------------- per t-chunk: QKV proj + attention ------------------
            for c in range(NTC):
                if c == 0:
                    # On-chip transpose: x rows 0..511 (critical path).
                    xT = slabp.tile([P, ND, TC], BF16, tag="slab", name="xT0")
                    for tb in range(TC // P):
                        natx = natp.tile([P, D], BF16, tag="nat", name=f"natx{tb}")
                        nc.gpsimd.dma_start(natx[:], x_ext[tb * P : (tb + 1) * P, :])
                        for g in range(0, ND, 4):
                            pst = psump.tile(
                                [P, 4 * P], BF16, tag="tr", bufs=2, name=f"ptrx{tb}{g}"
                            )
                            for j in range(4):
                                nc.tensor.transpose(
                                    pst[:, j * P : (j + 1) * P],
                                    natx[:, (g + j) * P : (g + j + 1) * P],
                                    ident[:],
                                )
                            nc.vector.tensor_copy(
                                xT[:, g : g + 4, tb * P : (tb + 1) * P],
                                pst[:].rearrange("p (g t) -> p g t", g=4),
                            )
                else:
                    xT = slabp.tile([P, ND, TC], BF16, tag="slab", name=f"xT{c}")
                    nc.sync.dma_start(
                        xT[:], x_bf[c * TC : (c + 1) * TC, :], transpose=True
                    )

                qTc = qtcp.tile([P, HL, TC], BF16, tag="qTc", name=f"qTc{c}")

                for ot in range(HL + 2):  # 0..7 = q heads, 8 = k, 9 = v
                    ps = psump.tile(
                        [P, TC], F32, tag="mm512", bufs=2, name=f"psqkv{c}_{ot}"
                    )
                    for dt in range(ND):
                        if ot < HL:
                            lhsT = wqT[:, dt, ot * P : (ot + 1) * P]
                        elif ot == HL:
                            lhsT = wkT[:, dt, :]
                        else:
                            lhsT = wvT[:, dt, :]
                        nc.tensor.matmul(
                            ps[:],
                            lhsT,
                            xT[:, dt, :],
                            start=(dt == 0),
                            stop=(dt == ND - 1),
                        )
                    if ot < HL:
                        nc.vector.tensor_scalar_add(
                            qTc[:, ot, :], ps[:], bq_sb[:, ot : ot + 1]
                        )
                    elif ot == HL:
                        nc.vector.tensor_scalar_add(
                            kT[:, c * TC : (c + 1) * TC], ps[:], bk_sb[:]
                        )
                    else:
                        vt = tmpp.tile([P, TC], BF16, tag="vt", bufs=1, name=f"vt{c}")
                        nc.vector.tensor_scalar_add(vt[:], ps[:], bv_sb[:])
                        pstv = psump.tile(
                            [P, 4 * P], BF16, tag="tr", bufs=2, name=f"pstv{c}"
                        )
                        for j in range(TC // P):
                            nc.tensor.transpose(
                                pstv[:, j * P : (j + 1) * P],
                                vt[:, j * P : (j + 1) * P],
                                ident[:],
                            )
                        nc.vector.tensor_copy(
                            vaug[:, c * (TC // P) : (c + 1) * (TC // P), 0:HD],
                            pstv[:].rearrange("p (g t) -> p g t", g=4),
                        )

                # After chunk-0 compute is queued, enqueue the deferred casts:
                # x chunks 1-3 first (needed soonest), then wo.
                if c == 0:
                    cast_to_scratch(x_ext, x_bf, TC, T, "x")
                    cast_to_scratch(wo_ext, wo_bf, 0, OL, "wo")

                # Attention for all local heads on this q-chunk.
                attnT_c = atcp.tile([P, HL, TC], BF16, tag="atc", name=f"attnT{c}")
                nkt = (c + 1) * (TC // P)  # causal: k-tiles 0..nkt-1
                for h in range(HL):
                    att_ps = [
                        psump.tile(
                            [P, HD + 1], F32, tag="attn", bufs=4, name=f"att{c}_{h}_{qt}"
                        )
                        for qt in range(TC // P)
                    ]
                    for kt in range(nkt):
                        ps_s = psump.tile(
                            [P, TC], F32, tag="mm512", bufs=2, name=f"pss{c}_{h}_{kt}"
                        )
                        nc.tensor.matmul(
                            ps_s[:],
                            kT[:, kt * P : (kt + 1) * P],
                            qTc[:, h, :],
                            start=True,
                            stop=True,
                        )
                        es = esbp.tile([P, TC], BF16, tag="esb", name=f"es{c}_{h}_{kt}")
                        nc.scalar.activation(
                            es[:],
                            ps_s[:],
                            mybir.ActivationFunctionType.Exp,
                            scale=SCALE,
                        )
                        if kt >= nkt - 4:
                            # Diagonal-band tile: zero weights where k > q.
                            nc.vector.tensor_tensor(
                                es[:], es[:], masks[kt - (nkt - 4)][:],
                                mybir.AluOpType.mult,
                            )
                        for qt in range(TC // P):
                            tqi = c * (TC // P) + qt
                            if kt > tqi:
                                continue
                            nc.tensor.matmul(
                                att_ps[qt][:],
                                es[:, qt * P : (qt + 1) * P],
                                vaug[:, kt, :],
                                start=(kt == 0),
                                stop=(kt == tqi),
                            )
                    psta = psump.tile(
                        [P, 4 * P], BF16, tag="tr", bufs=2, name=f"psta{c}{h}"
                    )
                    for qt in range(TC // P):
                        recip = tmpp.tile([P, 1], F32, tag="recip", bufs=2, name=f"rc{c}{h}{qt}")
                        nc.vector.reciprocal(recip[:], att_ps[qt][:, HD : HD + 1])
                        attn_sb = tmpp.tile(
                            [P, P], BF16, tag="attn_sb", bufs=2, name=f"asb{c}{h}{qt}"
                        )
                        nc.vector.tensor_scalar_mul(
                            attn_sb[:], att_ps[qt][:, 0:HD], recip[:]
                        )
                        nc.tensor.transpose(
                            psta[:, qt * P : (qt + 1) * P], attn_sb[:], ident[:]
                        )
                    nc.vector.tensor_copy(attnT_c[:, h, :], psta[:])

                # Ship this chunk's attn^T and AllGather it within the group.
                nc.sync.dma_start(
                    cc_in[c][:, :].rearrange("(h p) t -> p h t", p=P), attnT_c[:]
                )
                nc.gpsimd.collective_compute(
                    "AllGather",
                    mybir.AluOpType.bypass,
                    replica_groups=[[0, 1, 2, 3], [4, 5, 6, 7]],
                    ins=[cc_in[c][:, :].opt()],
                    outs=[cc_g[c][:, :].opt()],
                )

            # ---------------- Output projection -------------------------------
            woT = wpool.tile([P, ND, OL], BF16, tag="bigw", name="woT")
            nc.sync.dma_start(woT[:], wo_bf[:, :], transpose=True)

            for c in range(NTC):
                gT = slabp.tile([P, ND, TC], BF16, tag="slab", name=f"gT{c}")
                for ot in range(ND):
                    r, lh = divmod(ot, HL)
                    nc.sync.dma_start(
                        gT[:, ot, :],
                        cc_g[c][r * OL + lh * P : r * OL + (lh + 1) * P, :],
                    )
                for tt in range(TC // P):
                    for dc in range(OL // TC):
                        ps = psump.tile(
                            [P, TC], F32, tag="mm512", bufs=2, name=f"pso{c}_{tt}_{dc}"
                        )
                        for ot in range(ND):
                            nc.tensor.matmul(
                                ps[:],
                                gT[:, ot, tt * P : (tt + 1) * P],
                                woT[:, ot, dc * TC : (dc + 1) * TC],
                                start=(ot == 0),
                                stop=(ot == ND - 1),
                            )
                        osb = tmpp.tile([P, TC], F32, tag="osb", bufs=1, name=f"osb{c}{tt}{dc}")
                        nc.vector.tensor_tensor(
                            osb[:],
                            ps[:],
                            bo_bc[:, dc * TC : (dc + 1) * TC],
                            mybir.AluOpType.add,
                        )
                        nc.sync.dma_start(
                            out_ext[
                                c * TC + tt * P : c * TC + (tt + 1) * P,
                                dc * TC : (dc + 1) * TC,
                            ],
                            osb[:],
                        )

    nc.compile()
    return nc


def kernel(x, wq_w, wq_b, wk_w, wk_b, wv_w, wv_b, wo_w, wo_b):
    global LAST_RESULT
    if "nc" not in _CACHE:
        _CACHE["nc"] = build_nc()
    nc = _CACHE["nc"]

    f32 = np.float32
    x = np.asarray(x, f32)
    in_maps = []
    for c in range(NCORES):
        b, g = divmod(c, TPG)
        sl = slice(OL * g, OL * (g + 1))
        in_maps.append(
            {
                "x": np.ascontiguousarray(x[b]),
                "wq": np.ascontiguousarray(np.asarray(wq_w, f32)[sl]),
                "bq": np.ascontiguousarray(np.asarray(wq_b, f32)[sl]),
                "wk": np.ascontiguousarray(np.asarray(wk_w, f32)),
                "bk": np.ascontiguousarray(np.asarray(wk_b, f32)),
                "wv": np.ascontiguousarray(np.asarray(wv_w, f32)),
                "bv": np.ascontiguousarray(np.asarray(wv_b, f32)),
                "wo": np.ascontiguousarray(np.asarray(wo_w, f32)[sl]),
                "bo": np.ascontiguousarray(np.asarray(wo_b, f32)[sl]),
            }
        )

    res = run_bass_kernel_spmd(nc, in_maps, core_ids=list(range(NCORES)))
    LAST_RESULT = res

    out = np.empty((B, T, D), dtype=f32)
    for c in range(NCORES):
        b, g = divmod(c, TPG)
        out[b, :, OL * g : OL * (g + 1)] = res.results[c]["out"]
    return out


# revision 16
# speedup vs baseline: 1.1625x; 1.0312x over previous
"""Distributed MQA causal attention for TRN2 (8 NeuronCores).

Sharding: 8 cores = 2 (batch) x 4 (head-group tensor parallel).
Core c handles batch b=c//4, head group g=c%4 (8 heads, o-slice of 1024).
KV projection is replicated within a batch group.  After attention, the
per-core attn^T chunks are AllGather-ed (groups of 4) and each core computes
a 1024-wide column slice of the output projection.

All matmuls run in bf16 (f32 accumulation in PSUM).  The critical-path
operands (wq/wk/wv, x chunk 0) are cast f32->bf16 during the DMA load and
transposed on-chip with the (otherwise idle) TensorE so the first GEMM can
start after ~100us; the bulkier late operands (x chunks 1-3, wo) take the
DRAM bf16-bounce + DMA-transpose-xbar path, which overlaps compute.
"""

import numpy as np

import concourse.bass as bass
import concourse.mybir as mybir
import concourse.tile as tile
from concourse import bacc
from concourse.bass_utils import run_bass_kernel_spmd
from concourse.masks import make_identity

# Problem shape (hardcoded; kernel.py must be self-contained).
B, T, D = 2, 2048, 4096
H, HD = 32, 128
NCORES, TPG = 8, 4
HL = H // TPG            # 8 local heads per core
OL = HL * HD             # 1024 local q/o dims per core
P = 128
TC = 512                 # t-chunk width (moving-dim of the big GEMMs)
NTC = T // TC            # 4
ND = D // P              # 32 contraction tiles for D
NT = T // P              # 16 k-tiles
SCALE = float(1.0 / np.sqrt(HD))

BF16 = mybir.dt.bfloat16
F32 = mybir.dt.float32

_CACHE = {}
LAST_RESULT = None  # BassKernelResults of the most recent run (for test harness)


def build_nc():
    nc = bacc.Bacc(None, target_bir_lowering=False, num_devices=NCORES)

    x_ext = nc.declare_dram_parameter("x", [T, D], F32, isOutput=False)
    wq_ext = nc.declare_dram_parameter("wq", [OL, D], F32, isOutput=False)
    bq_ext = nc.declare_dram_parameter("bq", [OL], F32, isOutput=False)
    wk_ext = nc.declare_dram_parameter("wk", [HD, D], F32, isOutput=False)
    bk_ext = nc.declare_dram_parameter("bk", [HD], F32, isOutput=False)
    wv_ext = nc.declare_dram_parameter("wv", [HD, D], F32, isOutput=False)
    bv_ext = nc.declare_dram_parameter("bv", [HD], F32, isOutput=False)
    wo_ext = nc.declare_dram_parameter("wo", [OL, D], F32, isOutput=False)
    bo_ext = nc.declare_dram_parameter("bo", [OL], F32, isOutput=False)
    out_ext = nc.declare_dram_parameter("out", [T, OL], F32, isOutput=True)

    with tile.TileContext(nc) as tc:
        with (
            tc.tile_pool(name="consts", bufs=1) as consts,
            tc.tile_pool(name="wpool", bufs=1) as wpool,
            tc.tile_pool(name="wsmall", bufs=1) as wsmall,
            tc.tile_pool(name="slab", bufs=2) as slabp,
            tc.tile_pool(name="nat", bufs=2) as natp,
            tc.tile_pool(name="big", bufs=1) as bigp,
            tc.tile_pool(name="qtc", bufs=2) as qtcp,
            tc.tile_pool(name="atc", bufs=1) as atcp,
            tc.tile_pool(name="esb", bufs=3) as esbp,
            tc.tile_pool(name="tmp", bufs=3) as tmpp,
            tc.tile_pool(name="psum", bufs=1, space="PSUM") as psump,
            tc.tile_pool(name="dram", bufs=1, space="DRAM") as dram,
        ):
            # ---- Constants (tiny, first so gpsimd builds them before casts)
            ident = consts.tile([P, P], BF16)
            make_identity(nc, ident[:])
            # Causal 0/1 masks for the 4 diagonal-band offsets:
            # masks[i][x, y] = 1 if y - x - 128*i >= 0 else 0
            masks = []
            for i in range(4):
                m = consts.tile([P, TC], BF16, name=f"mask{i}")
                nc.gpsimd.memset(m[:], 1.0)
                nc.gpsimd.affine_select(
                    out=m[:],
                    in_=m[:],
                    pattern=[[1, TC]],
                    compare_op=mybir.AluOpType.is_ge,
                    fill=0.0,
                    base=-P * i,
                    channel_multiplier=-1,
                )
                masks.append(m)
            bq_sb = consts.tile([P, HL], F32)
            nc.sync.dma_start(bq_sb[:], bq_ext[:].rearrange("(o p) -> p o", p=P))
            bk_sb = consts.tile([P, 1], F32)
            nc.sync.dma_start(bk_sb[:], bk_ext[:].rearrange("(o p) -> p o", p=P))
            bv_sb = consts.tile([P, 1], F32)
            nc.sync.dma_start(bv_sb[:], bv_ext[:].rearrange("(o p) -> p o", p=P))
            bo_row = natp.tile([1, OL], BF16, tag="nat", name="bo_row")
            nc.gpsimd.dma_start(bo_row[:], bo_ext[None, :])
            bo_bc = consts.tile([P, OL], BF16)
            nc.gpsimd.partition_broadcast(bo_bc[:], bo_row[:])

            # Persistent attention operands.
            kT = bigp.tile([P, T], BF16, name="kT")               # [hd, t]
            vaug = bigp.tile([P, NT, HD + 1], BF16, name="vaug")  # [tk, kt, 129]
            nc.vector.memset(vaug[:, :, HD : HD + 1], 1.0)

            # ---- On-chip cast+transpose for critical-path operands ----------
            # src_ext [rows, D] f32 -> dstT[:, dt, col0 + 128*block] bf16
            def load_T_onchip(src_ext, rows, dstT, col0, what):
                for blk in range(rows // P):
                    nat = natp.tile([P, D], BF16, tag="nat", name=f"nat_{what}{blk}")
                    nc.gpsimd.dma_start(nat[:], src_ext[blk * P : (blk + 1) * P, :])
                    for g in range(0, ND, 4):
                        pst = psump.tile(
                            [P, 4 * P], BF16, tag="tr", bufs=2, name=f"ptr_{what}{blk}{g}"
                        )
                        for j in range(4):
                            nc.tensor.transpose(
                                pst[:, j * P : (j + 1) * P],
                                nat[:, (g + j) * P : (g + j + 1) * P],
                                ident[:],
                            )
                        nc.vector.tensor_copy(
                            dstT[:, g : g + 4, col0 + blk * P : col0 + (blk + 1) * P],
                            pst[:].rearrange("p (g t) -> p g t", g=4),
                        )

            wqT = wpool.tile([P, ND, OL], BF16, tag="bigw", name="wqT")
            wkT = wsmall.tile([P, ND, HD], BF16, name="wkT")
            wvT = wsmall.tile([P, ND, HD], BF16, name="wvT")

            # ---- DRAM bf16 scratch for the late operands (x1-3, wo) ---------
            x_bf = dram.tile([T, D], BF16)
            wo_bf = dram.tile([OL, D], BF16)

            def cast_to_scratch(src_ext, dst, r0, r1, what):
                step = P
                for i, r in enumerate(range(r0, r1, step)):
                    t_ = natp.tile([P, D], BF16, tag="nat", name=f"cs_{what}{i}")
                    nc.gpsimd.dma_start(t_[:], src_ext[r : r + step, :])
                    nc.sync.dma_start(dst[r : r + step, :], t_[:])

            # AllGather buffers, one per t-chunk (column-sliced attn^T).
            cc_in = [dram.tile([OL, TC], BF16, name=f"cc_in{c}") for c in range(NTC)]
            cc_g = [
                dram.tile([TPG * OL, TC], BF16, name=f"cc_g{c}") for c in range(NTC)
            ]

            # ------------- per t-chunk: QKV proj + attention ------------------
            for c in range(NTC):
                if c == 0:
                    # On-chip transpose: x rows 0..511 (critical path).
                    xT = slabp.tile([P, ND, TC], BF16, tag="slab", name="xT0")
                    for tb in range(TC // P):
                        natx = natp.tile([P, D], BF16, tag="nat", name=f"natx{tb}")
                        nc.gpsimd.dma_start(natx[:], x_ext[tb * P : (tb + 1) * P, :])
                        for g in range(0, ND, 4):
                            pst = psump.tile(
                                [P, 4 * P], BF16, tag="tr", bufs=2, name=f"ptrx{tb}{g}"
                            )
                            for j in range(4):
                                nc.tensor.transpose(
                                    pst[:, j * P : (j + 1) * P],
                                    natx[:, (g + j) * P : (g + j + 1) * P],
                                    ident[:],
                                )
                            nc.vector.tensor_copy(
                                xT[:, g : g + 4, tb * P : (tb + 1) * P],
                                pst[:].rearrange("p (g t) -> p g t", g=4),
                            )
                    load_T_onchip(wq_ext, OL, wqT, 0, "wq")
                    load_T_onchip(wk_ext, HD, wkT, 0, "wk")
                    load_T_onchip(wv_ext, HD, wvT, 0, "wv")
                else:
                    xT = slabp.tile([P, ND, TC], BF16, tag="slab", name=f"xT{c}")
                    nc.sync.dma_start(
                        xT[:], x_bf[c * TC : (c + 1) * TC, :], transpose=True
                    )

                qTc = qtcp.tile([P, HL, TC], BF16, tag="qTc", name=f"qTc{c}")

                for ot in range(HL + 2):  # 0..7 = q heads, 8 = k, 9 = v
                    ps = psump.tile(
                        [P, TC], F32, tag="mm512", bufs=2, name=f"psqkv{c}_{ot}"
                    )
                    for dt in range(ND):
                        if ot < HL:
                            lhsT = wqT[:, dt, ot * P : (ot + 1) * P]
                        elif ot == HL:
                            lhsT = wkT[:, dt, :]
                        else:
                            lhsT = wvT[:, dt, :]
                        nc.tensor.matmul(
                            ps[:],
                            lhsT,
                            xT[:, dt, :],
                            start=(dt == 0),
                            stop=(dt == ND - 1),
                        )
                    if ot < HL:
                        nc.vector.tensor_scalar_add(
                            qTc[:, ot, :], ps[:], bq_sb[:, ot : ot + 1]
                        )
                    elif ot == HL:
                        nc.vector.tensor_scalar_add(
                            kT[:, c * TC : (c + 1) * TC], ps[:], bk_sb[:]
                        )
                    else:
                        vt = tmpp.tile([P, TC], BF16, tag="vt", bufs=1, name=f"vt{c}")
                        nc.vector.tensor_scalar_add(vt[:], ps[:], bv_sb[:])
                        pstv = psump.tile(
                            [P, 4 * P], BF16, tag="tr", bufs=2, name=f"pstv{c}"
                        )
                        for j in range(TC // P):
                            nc.tensor.transpose(
                                pstv[:, j * P : (j + 1) * P],
                                vt[:, j * P : (j + 1) * P],
                                ident[:],
                            )
                        nc.vector.tensor_copy(
                            vaug[:, c * (TC // P) : (c + 1) * (TC // P), 0:HD],
                            pstv[:].rearrange("p (g t) -> p g t", g=4),
                        )

                # After chunk-0 compute is queued, enqueue the deferred casts:
                # x chunks 1-3 first (needed soonest), then wo.
                if c == 0:
                    cast_to_scratch(x_ext, x_bf, TC, T, "x")
                    cast_to_scratch(wo_ext, wo_bf, 0, OL, "wo")


                # Attention for all local heads on this q-chunk.
                attnT_c = atcp.tile([P, HL, TC], BF16, tag="atc", name=f"attnT{c}")
                nkt = (c + 1) * (TC // P)  # causal: k-tiles 0..nkt-1
                for h in range(HL):
                    att_ps = [
                        psump.tile(
                            [P, HD + 1], F32, tag="attn", bufs=4, name=f"att{c}_{h}_{qt}"
                        )
                        for qt in range(TC // P)
                    ]
                    for kt in range(nkt):
                        ps_s = psump.tile(
                            [P, TC], F32, tag="mm512", bufs=2, name=f"pss{c}_{h}_{kt}"
                        )
                        nc.tensor.matmul(
                            ps_s[:],
                            kT[:, kt * P : (kt + 1) * P],
                            qTc[:, h, :],
                            start=True,
                            stop=True,
                        )
                        es = esbp.tile([P, TC], BF16, tag="esb", name=f"es{c}_{h}_{kt}")
                        nc.scalar.activation(
                            es[:],
                            ps_s[:],
                            mybir.ActivationFunctionType.Exp,
                            scale=SCALE,
                        )
                        if kt >= nkt - 4:
                            # Diagonal-band tile: zero weights where k > q.
                            nc.vector.tensor_tensor(
                                es[:], es[:], masks[kt - (nkt - 4)][:],
                                mybir.AluOpType.mult,
                            )
                        for qt in range(TC // P):
                            tqi = c * (TC // P) + qt
                            if kt > tqi:
                                continue
                            nc.tensor.matmul(
                                att_ps[qt][:],
                                es[:, qt * P : (qt + 1) * P],
                                vaug[:, kt, :],
                                start=(kt == 0),
                                stop=(kt == tqi),
                            )
                    psta = psump.tile(
                        [P, 4 * P], BF16, tag="tr", bufs=2, name=f"psta{c}{h}"
                    )
                    for qt in range(TC // P):
                        recip = tmpp.tile([P, 1], F32, tag="recip", bufs=2, name=f"rc{c}{h}{qt}")
                        nc.vector.reciprocal(recip[:], att_ps[qt][:, HD : HD + 1])
                        attn_sb = tmpp.tile(
                            [P, P], BF16, tag="attn_sb", bufs=2, name=f"asb{c}{h}{qt}"
                        )
                        nc.vector.tensor_scalar_mul(
                            attn_sb[:], att_ps[qt][:, 0:HD], recip[:]
                        )
                        nc.tensor.transpose(
                            psta[:, qt * P : (qt + 1) * P], attn_sb[:], ident[:]
                        )
                    nc.vector.tensor_copy(attnT_c[:, h, :], psta[:])

                # Ship this chunk's attn^T and AllGather it within the group.
                nc.sync.dma_start(
                    cc_in[c][:, :].rearrange("(h p) t -> p h t", p=P), attnT_c[:]
                )
                nc.gpsimd.collective_compute(
                    "AllGather",
                    mybir.AluOpType.bypass,
                    replica_groups=[[0, 1, 2, 3], [4, 5, 6, 7]],
                    ins=[cc_in[c][:, :].opt()],
                    outs=[cc_g[c][:, :].opt()],
                )

            # ---------------- Output projection -------------------------------
            woT = wpool.tile([P, ND, OL], BF16, tag="bigw", name="woT")
            nc.sync.dma_start(woT[:, :, 0 : OL // 2], wo_bf[0 : OL // 2, :], transpose=True)
            nc.sync.dma_start(woT[:, :, OL // 2 : OL], wo_bf[OL // 2 : OL, :], transpose=True)

            for c in range(NTC):
                gT = slabp.tile([P, ND, TC], BF16, tag="slab", name=f"gT{c}")
                for ot in range(ND):
                    r, lh = divmod(ot, HL)
                    nc.sync.dma_start(
                        gT[:, ot, :],
                        cc_g[c][r * OL + lh * P : r * OL + (lh + 1) * P, :],
                    )
                for tt in range(TC // P):
                    for dc in range(OL // TC):
                        ps = psump.tile(
                            [P, TC], F32, tag="mm512", bufs=2, name=f"pso{c}_{tt}_{dc}"
                        )
                        for ot in range(ND):
                            nc.tensor.matmul(
                                ps[:],
                                gT[:, ot, tt * P : (tt + 1) * P],
                                woT[:, ot, dc * TC : (dc + 1) * TC],
                                start=(ot == 0),
                                stop=(ot == ND - 1),
                            )
                        osb = tmpp.tile([P, TC], F32, tag="osb", bufs=1, name=f"osb{c}{tt}{dc}")
                        nc.vector.tensor_tensor(
                            osb[:],
                            ps[:],
                            bo_bc[:, dc * TC : (dc + 1) * TC],
                            mybir.AluOpType.add,
                        )
                        nc.sync.dma_start(
                            out_ext[
                                c * TC + tt * P : c * TC + (tt + 1) * P,
                                dc * TC : (dc + 1) * TC,
                            ],
                            osb[:],
                        )

    nc.compile()
    return nc


def kernel(x, wq_w, wq_b, wk_w, wk_b, wv_w, wv_b, wo_w, wo_b):
    global LAST_RESULT
    if "nc" not in _CACHE:
        _CACHE["nc"] = build_nc()
    nc = _CACHE["nc"]

    f32 = np.float32
    x = np.asarray(x, f32)
    in_maps = []
    for c in range(NCORES):
        b, g = divmod(c, TPG)
        sl = slice(OL * g, OL * (g + 1))
        in_maps.append(
            {
                "x": np.ascontiguousarray(x[b]),
                "wq": np.ascontiguousarray(np.asarray(wq_w, f32)[sl]),
                "bq": np.ascontiguousarray(np.asarray(wq_b, f32)[sl]),
                "wk": np.ascontiguousarray(np.asarray(wk_w, f32)),
                "bk": np.ascontiguousarray(np.asarray(wk_b, f32)),
                "wv": np.ascontiguousarray(np.asarray(wv_w, f32)),
                "bv": np.ascontiguousarray(np.asarray(wv_b, f32)),
                "wo": np.ascontiguousarray(np.asarray(wo_w, f32)[sl]),
                "bo": np.ascontiguousarray(np.asarray(wo_b, f32)[sl]),
            }
        )

    res = run_bass_kernel_spmd(nc, in_maps, core_ids=list(range(NCORES)))
    LAST_RESULT = res

    out = np.empty((B, T, D), dtype=f32)
    for c in range(NCORES):
        b, g = divmod(c, TPG)
        out[b, :, OL * g : OL * (g + 1)] = res.results[c]["out"]
    return out


# revision 18
# speedup vs baseline: 1.1919x; 1.0253x over previous
"""Distributed MQA causal attention for TRN2 (8 NeuronCores).

Sharding: 8 cores = 2 (batch) x 4 (head-group tensor parallel).
Core c handles batch b=c//4, head group g=c%4 (8 heads, o-slice of 1024).
KV projection is replicated within a batch group.  After attention, the
per-core attn^T chunks are AllGather-ed (groups of 4) and each core computes
a 1024-wide column slice of the output projection.

All matmuls run in bf16 (f32 accumulation in PSUM).  The critical-path
operands (wq/wk/wv, x chunk 0) are cast f32->bf16 during the DMA load and
transposed on-chip with the (otherwise idle) TensorE so the first GEMM can
start after ~100us; the bulkier late operands (x chunks 1-3, wo) take the
DRAM bf16-bounce + DMA-transpose-xbar path, which overlaps compute.
"""

import numpy as np

import concourse.bass as bass
import concourse.mybir as mybir
import concourse.tile as tile
from concourse import bacc
from concourse.bass_utils import run_bass_kernel_spmd
from concourse.masks import make_identity

# Problem shape (hardcoded; kernel.py must be self-contained).
B, T, D = 2, 2048, 4096
H, HD = 32, 128
NCORES, TPG = 8, 4
HL = H // TPG            # 8 local heads per core
OL = HL * HD             # 1024 local q/o dims per core
P = 128
TC = 512                 # t-chunk width (moving-dim of the big GEMMs)
NTC = T // TC            # 4
ND = D // P              # 32 contraction tiles for D
NT = T // P              # 16 k-tiles
SCALE = float(1.0 / np.sqrt(HD))

BF16 = mybir.dt.bfloat16
F32 = mybir.dt.float32

_CACHE = {}
LAST_RESULT = None  # BassKernelResults of the most recent run (for test harness)


def build_nc():
    nc = bacc.Bacc(None, target_bir_lowering=False, num_devices=NCORES)

    x_ext = nc.declare_dram_parameter("x", [T, D], F32, isOutput=False)
    wq_ext = nc.declare_dram_parameter("wq", [OL, D], F32, isOutput=False)
    bq_ext = nc.declare_dram_parameter("bq", [OL], F32, isOutput=False)
    wk_ext = nc.declare_dram_parameter("wk", [HD, D], F32, isOutput=False)
    bk_ext = nc.declare_dram_parameter("bk", [HD], F32, isOutput=False)
    wv_ext = nc.declare_dram_parameter("wv", [HD, D], F32, isOutput=False)
    bv_ext = nc.declare_dram_parameter("bv", [HD], F32, isOutput=False)
    wo_ext = nc.declare_dram_parameter("wo", [OL, D], F32, isOutput=False)
    bo_ext = nc.declare_dram_parameter("bo", [OL], F32, isOutput=False)
    out_ext = nc.declare_dram_parameter("out", [T, OL], F32, isOutput=True)

    with tile.TileContext(nc) as tc:
        with (
            tc.tile_pool(name="consts", bufs=1) as consts,
            tc.tile_pool(name="wpool", bufs=1) as wpool,
            tc.tile_pool(name="wsmall", bufs=1) as wsmall,
            tc.tile_pool(name="slab", bufs=2) as slabp,
            tc.tile_pool(name="nat", bufs=2) as natp,
            tc.tile_pool(name="big", bufs=1) as bigp,
            tc.tile_pool(name="qtc", bufs=2) as qtcp,
            tc.tile_pool(name="atc", bufs=1) as atcp,
            tc.tile_pool(name="esb", bufs=3) as esbp,
            tc.tile_pool(name="tmp", bufs=3) as tmpp,
            tc.tile_pool(name="psum", bufs=1, space="PSUM") as psump,
            tc.tile_pool(name="dram", bufs=1, space="DRAM") as dram,
        ):
            # ---- Constants (tiny, first so gpsimd builds them before casts)
            ident = consts.tile([P, P], BF16)
            make_identity(nc, ident[:])
            # Causal 0/1 masks for the 4 diagonal-band offsets:
            # masks[i][x, y] = 1 if y - x - 128*i >= 0 else 0
            masks = []
            for i in range(4):
                m = consts.tile([P, TC], BF16, name=f"mask{i}")
                nc.gpsimd.memset(m[:], 1.0)
                nc.gpsimd.affine_select(
                    out=m[:],
                    in_=m[:],
                    pattern=[[1, TC]],
                    compare_op=mybir.AluOpType.is_ge,
                    fill=0.0,
                    base=-P * i,
                    channel_multiplier=-1,
                )
                masks.append(m)
            bq_sb = consts.tile([P, HL], F32)
            nc.sync.dma_start(bq_sb[:], bq_ext[:].rearrange("(o p) -> p o", p=P))
            bk_sb = consts.tile([P, 1], F32)
            nc.sync.dma_start(bk_sb[:], bk_ext[:].rearrange("(o p) -> p o", p=P))
            bv_sb = consts.tile([P, 1], F32)
            nc.sync.dma_start(bv_sb[:], bv_ext[:].rearrange("(o p) -> p o", p=P))
            bo_row = natp.tile([1, OL], BF16, tag="nat", name="bo_row")
            nc.gpsimd.dma_start(bo_row[:], bo_ext[None, :])
            bo_bc = consts.tile([P, OL], BF16)
            nc.gpsimd.partition_broadcast(bo_bc[:], bo_row[:])

            # Persistent attention operands.
            kT = bigp.tile([P, T], BF16, name="kT")               # [hd, t]
            vaug = bigp.tile([P, NT, HD + 1], BF16, name="vaug")  # [tk, kt, 129]
            nc.vector.memset(vaug[:, :, HD : HD + 1], 1.0)

            # ---- On-chip cast+transpose for critical-path operands ----------
            # src_ext [rows, D] f32 -> dstT[:, dt, col0 + 128*block] bf16
            def load_T_onchip(src_ext, rows, dstT, col0, what):
                for blk in range(rows // P):
                    nat = natp.tile([P, D], BF16, tag="nat", name=f"nat_{what}{blk}")
                    nc.gpsimd.dma_start(nat[:], src_ext[blk * P : (blk + 1) * P, :])
                    for g in range(0, ND, 4):
                        pst = psump.tile(
                            [P, 4 * P], BF16, tag="attn", bufs=4, name=f"ptr_{what}{blk}{g}"
                        )
                        for j in range(4):
                            nc.tensor.transpose(
                                pst[:, j * P : (j + 1) * P],
                                nat[:, (g + j) * P : (g + j + 1) * P],
                                ident[:],
                            )
                        nc.vector.tensor_copy(
                            dstT[:, g : g + 4, col0 + blk * P : col0 + (blk + 1) * P],
                            pst[:].rearrange("p (g t) -> p g t", g=4),
                        )

            wqT = wpool.tile([P, ND, OL], BF16, tag="bigw", name="wqT")
            wkT = wsmall.tile([P, ND, HD], BF16, name="wkT")
            wvT = wsmall.tile([P, ND, HD], BF16, name="wvT")

            # ---- DRAM bf16 scratch for the late operands (x1-3, wo) ---------
            x_bf = dram.tile([T, D], BF16)
            wo_bf = dram.tile([OL, D], BF16)

            def cast_to_scratch(src_ext, dst, r0, r1, what):
                step = P
                for i, r in enumerate(range(r0, r1, step)):
                    t_ = natp.tile([P, D], BF16, tag="nat", name=f"cs_{what}{i}")
                    nc.gpsimd.dma_start(t_[:], src_ext[r : r + step, :])
                    nc.sync.dma_start(dst[r : r + step, :], t_[:])

            # AllGather buffers, one per t-chunk (column-sliced attn^T).
            cc_in = [dram.tile([OL, TC], BF16, name=f"cc_in{c}") for c in range(NTC)]
            cc_g = [
                dram.tile([TPG * OL, TC], BF16, name=f"cc_g{c}") for c in range(NTC)
            ]

            woT = wpool.tile([P, ND, OL], BF16, tag="bigw", name="woT")

            def emit_outproj(c):
                gT = slabp.tile([P, ND, TC], BF16, tag="slab", name=f"gT{c}")
                for ot in range(ND):
                    r, lh = divmod(ot, HL)
                    nc.sync.dma_start(
                        gT[:, ot, :],
                        cc_g[c][r * OL + lh * P : r * OL + (lh + 1) * P, :],
                    )
                for tt in range(TC // P):
                    for dc in range(OL // TC):
                        ps = psump.tile(
                            [P, TC], F32, tag="tr", bufs=2, name=f"pso{c}_{tt}_{dc}"
                        )
                        for ot in range(ND):
                            nc.tensor.matmul(
                                ps[:],
                                gT[:, ot, tt * P : (tt + 1) * P],
                                woT[:, ot, dc * TC : (dc + 1) * TC],
                                start=(ot == 0),
                                stop=(ot == ND - 1),
                            )
                        osb = tmpp.tile([P, TC], F32, tag="osb", bufs=1, name=f"osb{c}{tt}{dc}")
                        nc.vector.tensor_tensor(
                            osb[:],
                            ps[:],
                            bo_bc[:, dc * TC : (dc + 1) * TC],
                            mybir.AluOpType.add,
                        )
                        nc.sync.dma_start(
                            out_ext[
                                c * TC + tt * P : c * TC + (tt + 1) * P,
                                dc * TC : (dc + 1) * TC,
                            ],
                            osb[:],
                        )

            # ------------- per t-chunk: QKV proj + attention ------------------
            for c in range(NTC):
                if c == 0:
                    # On-chip transpose: x rows 0..511 (critical path).
                    xT = slabp.tile([P, ND, TC], BF16, tag="slab", name="xT0")
                    for tb in range(TC // P):
                        natx = natp.tile([P, D], BF16, tag="nat", name=f"natx{tb}")
                        nc.gpsimd.dma_start(natx[:], x_ext[tb * P : (tb + 1) * P, :])
                        for g in range(0, ND, 4):
                            pst = psump.tile(
                                [P, 4 * P], BF16, tag="attn", bufs=4, name=f"ptrx{tb}{g}"
                            )
                            for j in range(4):
                                nc.tensor.transpose(
                                    pst[:, j * P : (j + 1) * P],
                                    natx[:, (g + j) * P : (g + j + 1) * P],
                                    ident[:],
                                )
                            nc.vector.tensor_copy(
                                xT[:, g : g + 4, tb * P : (tb + 1) * P],
                                pst[:].rearrange("p (g t) -> p g t", g=4),
                            )
                    load_T_onchip(wq_ext, OL, wqT, 0, "wq")
                    load_T_onchip(wk_ext, HD, wkT, 0, "wk")
                    load_T_onchip(wv_ext, HD, wvT, 0, "wv")
                else:
                    xT = slabp.tile([P, ND, TC], BF16, tag="slab", name=f"xT{c}")
                    nc.sync.dma_start(
                        xT[:], x_bf[c * TC : (c + 1) * TC, :], transpose=True
                    )

                qTc = qtcp.tile([P, HL, TC], BF16, tag="qTc", name=f"qTc{c}")

                for ot in range(HL + 2):  # 0..7 = q heads, 8 = k, 9 = v
                    ps = psump.tile(
                        [P, TC], F32, tag="mm512", bufs=2, name=f"psqkv{c}_{ot}"
                    )
                    for dt in range(ND):
                        if ot < HL:
                            lhsT = wqT[:, dt, ot * P : (ot + 1) * P]
                        elif ot == HL:
                            lhsT = wkT[:, dt, :]
                        else:
                            lhsT = wvT[:, dt, :]
                        nc.tensor.matmul(
                            ps[:],
                            lhsT,
                            xT[:, dt, :],
                            start=(dt == 0),
                            stop=(dt == ND - 1),
                        )
                    if ot < HL:
                        nc.vector.tensor_scalar_add(
                            qTc[:, ot, :], ps[:], bq_sb[:, ot : ot + 1]
                        )
                    elif ot == HL:
                        nc.vector.tensor_scalar_add(
                            kT[:, c * TC : (c + 1) * TC], ps[:], bk_sb[:]
                        )
                    else:
                        vt = tmpp.tile([P, TC], BF16, tag="vt", bufs=1, name=f"vt{c}")
                        nc.vector.tensor_scalar_add(vt[:], ps[:], bv_sb[:])
                        pstv = psump.tile(
                            [P, 4 * P], BF16, tag="attn", bufs=4, name=f"pstv{c}"
                        )
                        for j in range(TC // P):
                            nc.tensor.transpose(
                                pstv[:, j * P : (j + 1) * P],
                                vt[:, j * P : (j + 1) * P],
                                ident[:],
                            )
                        nc.vector.tensor_copy(
                            vaug[:, c * (TC // P) : (c + 1) * (TC // P), 0:HD],
                            pstv[:].rearrange("p (g t) -> p g t", g=4),
                        )

                # After chunk-0 compute is queued, enqueue the deferred casts:
                # x chunks 1-3 first (needed soonest), then wo.
                if c == 0:
                    cast_to_scratch(x_ext, x_bf, TC, T, "x")
                    cast_to_scratch(wo_ext, wo_bf, 0, OL, "wo")


                if c == NTC - 1:
                    nc.sync.dma_start(
                        woT[:, :, 0 : OL // 2], wo_bf[0 : OL // 2, :], transpose=True
                    )
                    nc.sync.dma_start(
                        woT[:, :, OL // 2 : OL], wo_bf[OL // 2 : OL, :], transpose=True
                    )

                # Attention for all local heads on this q-chunk.
                attnT_c = atcp.tile([P, HL, TC], BF16, tag="atc", name=f"attnT{c}")
                nkt = (c + 1) * (TC // P)  # causal: k-tiles 0..nkt-1
                for h in range(HL):
                    att_ps = [
                        psump.tile(
                            [P, HD + 1], F32, tag="attn", bufs=4, name=f"att{c}_{h}_{qt}"
                        )
                        for qt in range(TC // P)
                    ]
                    for kt in range(nkt):
                        ps_s = psump.tile(
                            [P, TC], F32, tag="mm512", bufs=2, name=f"pss{c}_{h}_{kt}"
                        )
                        nc.tensor.matmul(
                            ps_s[:],
                            kT[:, kt * P : (kt + 1) * P],
                            qTc[:, h, :],
                            start=True,
                            stop=True,
                        )
                        es = esbp.tile([P, TC], BF16, tag="esb", name=f"es{c}_{h}_{kt}")
                        nc.scalar.activation(
                            es[:],
                            ps_s[:],
                            mybir.ActivationFunctionType.Exp,
                            scale=SCALE,
                        )
                        if kt >= nkt - 4:
                            # Diagonal-band tile: zero weights where k > q.
                            nc.vector.tensor_tensor(
                                es[:], es[:], masks[kt - (nkt - 4)][:],
                                mybir.AluOpType.mult,
                            )
                        for qt in range(TC // P):
                            tqi = c * (TC // P) + qt
                            if kt > tqi:
                                continue
                            nc.tensor.matmul(
                                att_ps[qt][:],
                                es[:, qt * P : (qt + 1) * P],
                                vaug[:, kt, :],
                                start=(kt == 0),
                                stop=(kt == tqi),
                            )
                    psta = psump.tile(
                        [P, 4 * P], BF16, tag="attn", bufs=4, name=f"psta{c}{h}"
                    )
                    for qt in range(TC // P):
                        recip = tmpp.tile([P, 1], F32, tag="recip", bufs=2, name=f"rc{c}{h}{qt}")
                        nc.vector.reciprocal(recip[:], att_ps[qt][:, HD : HD + 1])
                        attn_sb = tmpp.tile(
                            [P, P], BF16, tag="attn_sb", bufs=2, name=f"asb{c}{h}{qt}"
                        )
                        nc.vector.tensor_scalar_mul(
                            attn_sb[:], att_ps[qt][:, 0:HD], recip[:]
                        )
                        nc.tensor.transpose(
                            psta[:, qt * P : (qt + 1) * P], attn_sb[:], ident[:]
                        )
                    nc.vector.tensor_copy(attnT_c[:, h, :], psta[:])
                    if c == NTC - 1 and h == 3:
                        emit_outproj(0)
                    if c == NTC - 1 and h == 5:
                        emit_outproj(1)
                    if c == NTC - 1 and h == 7:
                        emit_outproj(2)

                # Ship this chunk's attn^T and AllGather it within the group.
                nc.sync.dma_start(
                    cc_in[c][:, :].rearrange("(h p) t -> p h t", p=P), attnT_c[:]
                )
                nc.gpsimd.collective_compute(
                    "AllGather",
                    mybir.AluOpType.bypass,
                    replica_groups=[[0, 1, 2, 3], [4, 5, 6, 7]],
                    ins=[cc_in[c][:, :].opt()],
                    outs=[cc_g[c][:, :].opt()],
                )
                if c == NTC - 1:
                    emit_outproj(3)

            # (output projection is emitted interleaved, see emit_outproj above)

    nc.compile()
    return nc


def kernel(x, wq_w, wq_b, wk_w, wk_b, wv_w, wv_b, wo_w, wo_b):
    global LAST_RESULT
    if "nc" not in _CACHE:
        _CACHE["nc"] = build_nc()
    nc = _CACHE["nc"]

    f32 = np.float32
    x = np.asarray(x, f32)
    in_maps = []
    for c in range(NCORES):
        b, g = divmod(c, TPG)
        sl = slice(OL * g, OL * (g + 1))
        in_maps.append(
            {
                "x": np.ascontiguousarray(x[b]),
                "wq": np.ascontiguousarray(np.asarray(wq_w, f32)[sl]),
                "bq": np.ascontiguousarray(np.asarray(wq_b, f32)[sl]),
                "wk": np.ascontiguousarray(np.asarray(wk_w, f32)),
                "bk": np.ascontiguousarray(np.asarray(wk_b, f32)),
                "wv": np.ascontiguousarray(np.asarray(wv_w, f32)),
                "bv": np.ascontiguousarray(np.asarray(wv_b, f32)),
                "wo": np.ascontiguousarray(np.asarray(wo_w, f32)[sl]),
                "bo": np.ascontiguousarray(np.asarray(wo_b, f32)[sl]),
            }
        )

    res = run_bass_kernel_spmd(nc, in_maps, core_ids=list(range(NCORES)))
    LAST_RESULT = res

    out = np.empty((B, T, D), dtype=f32)
    for c in range(NCORES):
        b, g = divmod(c, TPG)
        out[b, :, OL * g : OL * (g + 1)] = res.results[c]["out"]
    return out


# revision 19
# speedup vs baseline: 1.2058x; 1.0116x over previous
"""Distributed MQA causal attention for TRN2 (8 NeuronCores).

Sharding: 8 cores = 2 (batch) x 4 (head-group tensor parallel).
Core c handles batch b=c//4, head group g=c%4 (8 heads, o-slice of 1024).
KV projection is replicated within a batch group.  After attention, the
per-core attn^T chunks are AllGather-ed (groups of 4) and each core computes
a 1024-wide column slice of the output projection.

All matmuls run in bf16 (f32 accumulation in PSUM).  The critical-path
operands (wq/wk/wv, x chunk 0) are cast f32->bf16 during the DMA load and
transposed on-chip with the (otherwise idle) TensorE so the first GEMM can
start after ~100us; the bulkier late operands (x chunks 1-3, wo) take the
DRAM bf16-bounce + DMA-transpose-xbar path, which overlaps compute.
"""

import numpy as np

import concourse.bass as bass
import concourse.mybir as mybir
import concourse.tile as tile
from concourse import bacc
from concourse.bass_utils import run_bass_kernel_spmd
from concourse.masks import make_identity

# Problem shape (hardcoded; kernel.py must be self-contained).
B, T, D = 2, 2048, 4096
H, HD = 32, 128
NCORES, TPG = 8, 4
HL = H // TPG            # 8 local heads per core
OL = HL * HD             # 1024 local q/o dims per core
P = 128
TC = 512                 # t-chunk width (moving-dim of the big GEMMs)
NTC = T // TC            # 4
ND = D // P              # 32 contraction tiles for D
NT = T // P              # 16 k-tiles
SCALE = float(1.0 / np.sqrt(HD))

BF16 = mybir.dt.bfloat16
F32 = mybir.dt.float32

_CACHE = {}
LAST_RESULT = None  # BassKernelResults of the most recent run (for test harness)


def build_nc():
    nc = bacc.Bacc(None, target_bir_lowering=False, num_devices=NCORES)

    x_ext = nc.declare_dram_parameter("x", [T, D], F32, isOutput=False)
    wq_ext = nc.declare_dram_parameter("wq", [OL, D], F32, isOutput=False)
    bq_ext = nc.declare_dram_parameter("bq", [OL], F32, isOutput=False)
    wk_ext = nc.declare_dram_parameter("wk", [HD, D], F32, isOutput=False)
    bk_ext = nc.declare_dram_parameter("bk", [HD], F32, isOutput=False)
    wv_ext = nc.declare_dram_parameter("wv", [HD, D], F32, isOutput=False)
    bv_ext = nc.declare_dram_parameter("bv", [HD], F32, isOutput=False)
    wo_ext = nc.declare_dram_parameter("wo", [OL, D], F32, isOutput=False)
    bo_ext = nc.declare_dram_parameter("bo", [OL], F32, isOutput=False)
    out_ext = nc.declare_dram_parameter("out", [T, OL], F32, isOutput=True)

    with tile.TileContext(nc) as tc:
        with (
            tc.tile_pool(name="consts", bufs=1) as consts,
            tc.tile_pool(name="wpool", bufs=1) as wpool,
            tc.tile_pool(name="wsmall", bufs=1) as wsmall,
            tc.tile_pool(name="slab", bufs=2) as slabp,
            tc.tile_pool(name="nat", bufs=2) as natp,
            tc.tile_pool(name="big", bufs=1) as bigp,
            tc.tile_pool(name="qtc", bufs=2) as qtcp,
            tc.tile_pool(name="atc", bufs=1) as atcp,
            tc.tile_pool(name="esb", bufs=3) as esbp,
            tc.tile_pool(name="tmp", bufs=3) as tmpp,
            tc.tile_pool(name="psum", bufs=1, space="PSUM") as psump,
            tc.tile_pool(name="dram", bufs=1, space="DRAM") as dram,
        ):
            # ---- Constants (tiny, first so gpsimd builds them before casts)
            ident = consts.tile([P, P], BF16)
            make_identity(nc, ident[:])
            # Diagonal causal 0/1 mask: mask0[x, y] = 1 if y >= x else 0.
            # Band S-tiles are column-trimmed so every one reduces to this.
            mask0 = consts.tile([P, TC], BF16, name="mask0")
            nc.gpsimd.memset(mask0[:], 1.0)
            nc.gpsimd.affine_select(
                out=mask0[:],
                in_=mask0[:],
                pattern=[[1, TC]],
                compare_op=mybir.AluOpType.is_ge,
                fill=0.0,
                base=0,
                channel_multiplier=-1,
            )
            bq_sb = consts.tile([P, HL], F32)
            nc.sync.dma_start(bq_sb[:], bq_ext[:].rearrange("(o p) -> p o", p=P))
            bk_sb = consts.tile([P, 1], F32)
            nc.sync.dma_start(bk_sb[:], bk_ext[:].rearrange("(o p) -> p o", p=P))
            bv_sb = consts.tile([P, 1], F32)
            nc.sync.dma_start(bv_sb[:], bv_ext[:].rearrange("(o p) -> p o", p=P))
            bo_row = natp.tile([1, OL], BF16, tag="nat", name="bo_row")
            nc.gpsimd.dma_start(bo_row[:], bo_ext[None, :])
            bo_bc = consts.tile([P, OL], BF16)
            nc.gpsimd.partition_broadcast(bo_bc[:], bo_row[:])

            # Persistent attention operands.
            kT = bigp.tile([P, T], BF16, name="kT")               # [hd, t]
            vaug = bigp.tile([P, NT, HD + 1], BF16, name="vaug")  # [tk, kt, 129]
            nc.vector.memset(vaug[:, :, HD : HD + 1], 1.0)

            # ---- On-chip cast+transpose for critical-path operands ----------
            # src_ext [rows, D] f32 -> dstT[:, dt, col0 + 128*block] bf16
            def load_T_onchip(src_ext, rows, dstT, col0, what):
                for blk in range(rows // P):
                    nat = natp.tile([P, D], BF16, tag="nat", name=f"nat_{what}{blk}")
                    nc.gpsimd.dma_start(nat[:], src_ext[blk * P : (blk + 1) * P, :])
                    for g in range(0, ND, 4):
                        pst = psump.tile(
                            [P, 4 * P], BF16, tag="attn", bufs=4, name=f"ptr_{what}{blk}{g}"
                        )
                        for j in range(4):
                            nc.tensor.transpose(
                                pst[:, j * P : (j + 1) * P],
                                nat[:, (g + j) * P : (g + j + 1) * P],
                                ident[:],
                            )
                        nc.vector.tensor_copy(
                            dstT[:, g : g + 4, col0 + blk * P : col0 + (blk + 1) * P],
                            pst[:].rearrange("p (g t) -> p g t", g=4),
                        )

            wqT = wpool.tile([P, ND, OL], BF16, tag="bigw", name="wqT")
            wkT = wsmall.tile([P, ND, HD], BF16, name="wkT")
            wvT = wsmall.tile([P, ND, HD], BF16, name="wvT")

            # ---- DRAM bf16 scratch for the late operands (x1-3, wo) ---------
            x_bf = dram.tile([T, D], BF16)
            wo_bf = dram.tile([OL, D], BF16)

            def cast_to_scratch(src_ext, dst, r0, r1, what):
                step = P
                for i, r in enumerate(range(r0, r1, step)):
                    t_ = natp.tile([P, D], BF16, tag="nat", name=f"cs_{what}{i}")
                    nc.gpsimd.dma_start(t_[:], src_ext[r : r + step, :])
                    nc.sync.dma_start(dst[r : r + step, :], t_[:])

            # AllGather buffers, one per t-chunk (column-sliced attn^T).
            cc_in = [dram.tile([OL, TC], BF16, name=f"cc_in{c}") for c in range(NTC)]
            cc_g = [
                dram.tile([TPG * OL, TC], BF16, name=f"cc_g{c}") for c in range(NTC)
            ]

            woT = wpool.tile([P, ND, OL], BF16, tag="bigw", name="woT")

            def emit_outproj(c):
                gT = slabp.tile([P, ND, TC], BF16, tag="slab", name=f"gT{c}")
                for ot in range(ND):
                    r, lh = divmod(ot, HL)
                    nc.sync.dma_start(
                        gT[:, ot, :],
                        cc_g[c][r * OL + lh * P : r * OL + (lh + 1) * P, :],
                    )
                for tt in range(TC // P):
                    for dc in range(OL // TC):
                        ps = psump.tile(
                            [P, TC], F32, tag="tr", bufs=2, name=f"pso{c}_{tt}_{dc}"
                        )
                        for ot in range(ND):
                            nc.tensor.matmul(
                                ps[:],
                                gT[:, ot, tt * P : (tt + 1) * P],
                                woT[:, ot, dc * TC : (dc + 1) * TC],
                                start=(ot == 0),
                                stop=(ot == ND - 1),
                            )
                        osb = tmpp.tile([P, TC], F32, tag="osb", bufs=2, name=f"osb{c}{tt}{dc}")
                        nc.vector.tensor_tensor(
                            osb[:],
                            ps[:],
                            bo_bc[:, dc * TC : (dc + 1) * TC],
                            mybir.AluOpType.add,
                        )
                        nc.sync.dma_start(
                            out_ext[
                                c * TC + tt * P : c * TC + (tt + 1) * P,
                                dc * TC : (dc + 1) * TC,
                            ],
                            osb[:],
                        )

            # ------------- per t-chunk: QKV proj + attention ------------------
            for c in range(NTC):
                if c == 0:
                    # On-chip transpose: x rows 0..511 (critical path).
                    xT = slabp.tile([P, ND, TC], BF16, tag="slab", name="xT0")
                    for tb in range(TC // P):
                        natx = natp.tile([P, D], BF16, tag="nat", name=f"natx{tb}")
                        nc.gpsimd.dma_start(natx[:], x_ext[tb * P : (tb + 1) * P, :])
                        for g in range(0, ND, 4):
                            pst = psump.tile(
                                [P, 4 * P], BF16, tag="attn", bufs=4, name=f"ptrx{tb}{g}"
                            )
                            for j in range(4):
                                nc.tensor.transpose(
                                    pst[:, j * P : (j + 1) * P],
                                    natx[:, (g + j) * P : (g + j + 1) * P],
                                    ident[:],
                                )
                            nc.vector.tensor_copy(
                                xT[:, g : g + 4, tb * P : (tb + 1) * P],
                                pst[:].rearrange("p (g t) -> p g t", g=4),
                            )
                    load_T_onchip(wq_ext, OL, wqT, 0, "wq")
                    load_T_onchip(wk_ext, HD, wkT, 0, "wk")
                    load_T_onchip(wv_ext, HD, wvT, 0, "wv")
                else:
                    xT = slabp.tile([P, ND, TC], BF16, tag="slab", name=f"xT{c}")
                    nc.sync.dma_start(
                        xT[:], x_bf[c * TC : (c + 1) * TC, :], transpose=True
                    )

                qTc = qtcp.tile([P, HL, TC], BF16, tag="qTc", name=f"qTc{c}")

                for ot in range(HL + 2):  # 0..7 = q heads, 8 = k, 9 = v
                    ps = psump.tile(
                        [P, TC], F32, tag="mm512", bufs=2, name=f"psqkv{c}_{ot}"
                    )
                    for dt in range(ND):
                        if ot < HL:
                            lhsT = wqT[:, dt, ot * P : (ot + 1) * P]
                        elif ot == HL:
                            lhsT = wkT[:, dt, :]
                        else:
                            lhsT = wvT[:, dt, :]
                        nc.tensor.matmul(
                            ps[:],
                            lhsT,
                            xT[:, dt, :],
                            start=(dt == 0),
                            stop=(dt == ND - 1),
                        )
                    if ot < HL:
                        nc.vector.tensor_scalar_add(
                            qTc[:, ot, :], ps[:], bq_sb[:, ot : ot + 1]
                        )
                    elif ot == HL:
                        nc.vector.tensor_scalar_add(
                            kT[:, c * TC : (c + 1) * TC], ps[:], bk_sb[:]
                        )
                    else:
                        vt = tmpp.tile([P, TC], BF16, tag="vt", bufs=1, name=f"vt{c}")
                        nc.vector.tensor_scalar_add(vt[:], ps[:], bv_sb[:])
                        pstv = psump.tile(
                            [P, 4 * P], BF16, tag="attn", bufs=4, name=f"pstv{c}"
                        )
                        for j in range(TC // P):
                            nc.tensor.transpose(
                                pstv[:, j * P : (j + 1) * P],
                                vt[:, j * P : (j + 1) * P],
                                ident[:],
                            )
                        nc.vector.tensor_copy(
                            vaug[:, c * (TC // P) : (c + 1) * (TC // P), 0:HD],
                            pstv[:].rearrange("p (g t) -> p g t", g=4),
                        )

                # After chunk-0 compute is queued, enqueue the deferred casts:
                # x chunks 1-3 first (needed soonest), then wo.
                if c == 0:
                    cast_to_scratch(x_ext, x_bf, TC, T, "x")
                    cast_to_scratch(wo_ext, wo_bf, 0, OL, "wo")


                if c == NTC - 1:
                    nc.sync.dma_start(
                        woT[:, :, 0 : OL // 2], wo_bf[0 : OL // 2, :], transpose=True
                    )
                    nc.sync.dma_start(
                        woT[:, :, OL // 2 : OL], wo_bf[OL // 2 : OL, :], transpose=True
                    )

                # Attention for all local heads on this q-chunk.
                attnT_c = atcp.tile([P, HL, TC], BF16, tag="atc", name=f"attnT{c}")
                nkt = (c + 1) * (TC // P)  # causal: k-tiles 0..nkt-1
                for h in range(HL):
                    att_ps = [
                        psump.tile(
                            [P, HD + 1], F32, tag="attn", bufs=4, name=f"att{c}_{h}_{qt}"
                        )
                        for qt in range(TC // P)
                    ]
                    for kt in range(nkt):
                        # Band tiles only need columns tq >= (kt-4c)*128; after
                        # trimming, the causal pattern is always the diagonal.
                        off = (kt - (nkt - 4)) * P if kt >= nkt - 4 else 0
                        ne = TC - off
                        ps_s = psump.tile(
                            [P, TC], F32, tag="mm512", bufs=2, name=f"pss{c}_{h}_{kt}"
                        )
                        nc.tensor.matmul(
                            ps_s[:, :ne],
                            kT[:, kt * P : (kt + 1) * P],
                            qTc[:, h, off:TC],
                            start=True,
                            stop=True,
                        )
                        es = esbp.tile([P, TC], BF16, tag="esb", name=f"es{c}_{h}_{kt}")
                        nc.scalar.activation(
                            es[:, :ne],
                            ps_s[:, :ne],
                            mybir.ActivationFunctionType.Exp,
                            scale=SCALE,
                        )
                        if kt >= nkt - 4:
                            # Zero weights where k > q (pure diagonal after trim).
                            nc.vector.tensor_tensor(
                                es[:, :ne], es[:, :ne], mask0[:, :ne],
                                mybir.AluOpType.mult,
                            )
                        for qt in range(TC // P):
                            tqi = c * (TC // P) + qt
                            if kt > tqi:
                                continue
                            nc.tensor.matmul(
                                att_ps[qt][:],
                                es[:, qt * P - off : (qt + 1) * P - off],
                                vaug[:, kt, :],
                                start=(kt == 0),
                                stop=(kt == tqi),
                            )
                    psta = psump.tile(
                        [P, 4 * P], BF16, tag="attn", bufs=4, name=f"psta{c}{h}"
                    )
                    for qt in range(TC // P):
                        recip = tmpp.tile([P, 1], F32, tag="recip", bufs=2, name=f"rc{c}{h}{qt}")
                        nc.vector.reciprocal(recip[:], att_ps[qt][:, HD : HD + 1])
                        attn_sb = tmpp.tile(
                            [P, P], BF16, tag="attn_sb", bufs=2, name=f"asb{c}{h}{qt}"
                        )
                        nc.vector.tensor_scalar_mul(
                            attn_sb[:], att_ps[qt][:, 0:HD], recip[:]
                        )
                        nc.tensor.transpose(
                            psta[:, qt * P : (qt + 1) * P], attn_sb[:], ident[:]
                        )
                    nc.vector.tensor_copy(attnT_c[:, h, :], psta[:])
                    if c == NTC - 1 and h == 3:
                        emit_outproj(0)
                    if c == NTC - 1 and h == 5:
                        emit_outproj(1)
                    if c == NTC - 1 and h == 7:
                        emit_outproj(2)

                # Ship this chunk's attn^T and AllGather it within the group.
                nc.sync.dma_start(
                    cc_in[c][:, :].rearrange("(h p) t -> p h t", p=P), attnT_c[:]
                )
                nc.gpsimd.collective_compute(
                    "AllGather",
                    mybir.AluOpType.bypass,
                    replica_groups=[[0, 1, 2, 3], [4, 5, 6, 7]],
                    ins=[cc_in[c][:, :].opt()],
                    outs=[cc_g[c][:, :].opt()],
                )
                if c == NTC - 1:
                    emit_outproj(3)

            # (output projection is emitted interleaved, see emit_outproj above)

    nc.compile()
    return nc


def kernel(x, wq_w, wq_b, wk_w, wk_b, wv_w, wv_b, wo_w, wo_b):
    global LAST_RESULT
    if "nc" not in _CACHE:
        _CACHE["nc"] = build_nc()
    nc = _CACHE["nc"]

    f32 = np.float32
    x = np.asarray(x, f32)
    in_maps = []
    for c in range(NCORES):
        b, g = divmod(c, TPG)
        sl = slice(OL * g, OL * (g + 1))
        in_maps.append(
            {
                "x": np.ascontiguousarray(x[b]),
                "wq": np.ascontiguousarray(np.asarray(wq_w, f32)[sl]),
                "bq": np.ascontiguousarray(np.asarray(wq_b, f32)[sl]),
                "wk": np.ascontiguousarray(np.asarray(wk_w, f32)),
                "bk": np.ascontiguousarray(np.asarray(wk_b, f32)),
                "wv": np.ascontiguousarray(np.asarray(wv_w, f32)),
                "bv": np.ascontiguousarray(np.asarray(wv_b, f32)),
                "wo": np.ascontiguousarray(np.asarray(wo_w, f32)[sl]),
                "bo": np.ascontiguousarray(np.asarray(wo_b, f32)[sl]),
            }
        )

    res = run_bass_kernel_spmd(nc, in_maps, core_ids=list(range(NCORES)))
    LAST_RESULT = res

    out = np.empty((B, T, D), dtype=f32)
    for c in range(NCORES):
        b, g = divmod(c, TPG)
        out[b, :, OL * g : OL * (g + 1)] = res.results[c]["out"]
    return out


# revision 20
# speedup vs baseline: 1.2264x; 1.0171x over previous
"""Distributed MQA causal attention for TRN2 (8 NeuronCores).

Sharding: 8 cores = 2 (batch) x 4 (head-group tensor parallel).
Core c handles batch b=c//4, head group g=c%4 (8 heads, o-slice of 1024).
KV projection is replicated within a batch group.  After attention, the
per-core attn^T chunks are AllGather-ed (groups of 4) and each core computes
a 1024-wide column slice of the output projection.

All matmuls run in bf16 (f32 accumulation in PSUM).  The critical-path
operands (wq/wk/wv, x chunk 0) are cast f32->bf16 during the DMA load and
transposed on-chip with the (otherwise idle) TensorE so the first GEMM can
start after ~100us; the bulkier late operands (x chunks 1-3, wo) take the
DRAM bf16-bounce + DMA-transpose-xbar path, which overlaps compute.
"""

import numpy as np

import concourse.bass as bass
import concourse.mybir as mybir
import concourse.tile as tile
from concourse import bacc
from concourse.bass_utils import run_bass_kernel_spmd
from concourse.masks import make_identity

# Problem shape (hardcoded; kernel.py must be self-contained).
B, T, D = 2, 2048, 4096
H, HD = 32, 128
NCORES, TPG = 8, 4
HL = H // TPG            # 8 local heads per core
OL = HL * HD             # 1024 local q/o dims per core
P = 128
TC = 512                 # t-chunk width (moving-dim of the big GEMMs)
NTC = T // TC            # 4
ND = D // P              # 32 contraction tiles for D
NT = T // P              # 16 k-tiles
SCALE = float(1.0 / np.sqrt(HD))

BF16 = mybir.dt.bfloat16
F32 = mybir.dt.float32

_CACHE = {}
LAST_RESULT = None  # BassKernelResults of the most recent run (for test harness)


def build_nc():
    nc = bacc.Bacc(None, target_bir_lowering=False, num_devices=NCORES)

    x_ext = nc.declare_dram_parameter("x", [T, D], F32, isOutput=False)
    wq_ext = nc.declare_dram_parameter("wq", [OL, D], F32, isOutput=False)
    bq_ext = nc.declare_dram_parameter("bq", [OL], F32, isOutput=False)
    wk_ext = nc.declare_dram_parameter("wk", [HD, D], F32, isOutput=False)
    bk_ext = nc.declare_dram_parameter("bk", [HD], F32, isOutput=False)
    wv_ext = nc.declare_dram_parameter("wv", [HD, D], F32, isOutput=False)
    bv_ext = nc.declare_dram_parameter("bv", [HD], F32, isOutput=False)
    wo_ext = nc.declare_dram_parameter("wo", [OL, D], F32, isOutput=False)
    bo_ext = nc.declare_dram_parameter("bo", [OL], F32, isOutput=False)
    out_ext = nc.declare_dram_parameter("out", [T, OL], F32, isOutput=True)

    with tile.TileContext(nc) as tc:
        with (
            tc.tile_pool(name="consts", bufs=1) as consts,
            tc.tile_pool(name="wpool", bufs=1) as wpool,
            tc.tile_pool(name="wsmall", bufs=1) as wsmall,
            tc.tile_pool(name="slab", bufs=2) as slabp,
            tc.tile_pool(name="nat", bufs=2) as natp,
            tc.tile_pool(name="big", bufs=1) as bigp,
            tc.tile_pool(name="qtc", bufs=2) as qtcp,
            tc.tile_pool(name="atc", bufs=1) as atcp,
            tc.tile_pool(name="esb", bufs=3) as esbp,
            tc.tile_pool(name="tmp", bufs=3) as tmpp,
            tc.tile_pool(name="psum", bufs=1, space="PSUM") as psump,
            tc.tile_pool(name="dram", bufs=1, space="DRAM") as dram,
        ):
            # ---- Constants (tiny, first so gpsimd builds them before casts)
            ident = consts.tile([P, P], BF16)
            make_identity(nc, ident[:])
            # Diagonal causal 0/1 mask: mask0[x, y] = 1 if y >= x else 0.
            # Band S-tiles are column-trimmed so every one reduces to this.
            mask0 = consts.tile([P, TC], BF16, name="mask0")
            nc.gpsimd.memset(mask0[:], 1.0)
            nc.gpsimd.affine_select(
                out=mask0[:],
                in_=mask0[:],
                pattern=[[1, TC]],
                compare_op=mybir.AluOpType.is_ge,
                fill=0.0,
                base=0,
                channel_multiplier=-1,
            )
            bq_sb = consts.tile([P, HL], F32)
            nc.sync.dma_start(bq_sb[:], bq_ext[:].rearrange("(o p) -> p o", p=P))
            bk_sb = consts.tile([P, 1], F32)
            nc.sync.dma_start(bk_sb[:], bk_ext[:].rearrange("(o p) -> p o", p=P))
            bv_sb = consts.tile([P, 1], F32)
            nc.sync.dma_start(bv_sb[:], bv_ext[:].rearrange("(o p) -> p o", p=P))
            bo_row = natp.tile([1, OL], BF16, tag="nat", name="bo_row")
            nc.gpsimd.dma_start(bo_row[:], bo_ext[None, :])
            bo_bc = consts.tile([P, OL], BF16)
            nc.gpsimd.partition_broadcast(bo_bc[:], bo_row[:])

            # Persistent attention operands.
            kT = bigp.tile([P, T], BF16, name="kT")               # [hd, t]
            vaug = bigp.tile([P, NT, HD + 1], BF16, name="vaug")  # [tk, kt, 129]
            nc.vector.memset(vaug[:, :, HD : HD + 1], 1.0)

            # ---- On-chip cast+transpose for critical-path operands ----------
            # src_ext [rows, D] f32 -> dstT[:, dt, col0 + 128*block] bf16
            def load_T_onchip(src_ext, rows, dstT, col0, what):
                for blk in range(rows // P):
                    nat = natp.tile([P, D], BF16, tag="nat", name=f"nat_{what}{blk}")
                    nc.gpsimd.dma_start(nat[:], src_ext[blk * P : (blk + 1) * P, :])
                    for g in range(0, ND, 4):
                        pst = psump.tile(
                            [P, 4 * P], BF16, tag="attn", bufs=4, name=f"ptr_{what}{blk}{g}"
                        )
                        for j in range(4):
                            nc.tensor.transpose(
                                pst[:, j * P : (j + 1) * P],
                                nat[:, (g + j) * P : (g + j + 1) * P],
                                ident[:],
                            )
                        nc.vector.tensor_copy(
                            dstT[:, g : g + 4, col0 + blk * P : col0 + (blk + 1) * P],
                            pst[:].rearrange("p (g t) -> p g t", g=4),
                        )

            wqT = wpool.tile([P, ND, OL], BF16, tag="bigw", name="wqT")
            wkT = wsmall.tile([P, ND, HD], BF16, name="wkT")
            wvT = wsmall.tile([P, ND, HD], BF16, name="wvT")

            # ---- DRAM bf16 scratch for the late operands (x1-3, wo) ---------
            x_bf = dram.tile([T, D], BF16)
            wo_bf = dram.tile([OL, D], BF16)

            def cast_to_scratch(src_ext, dst, r0, r1, what):
                step = P
                for i, r in enumerate(range(r0, r1, step)):
                    t_ = natp.tile([P, D], BF16, tag="nat", name=f"cs_{what}{i}")
                    nc.gpsimd.dma_start(t_[:], src_ext[r : r + step, :])
                    nc.sync.dma_start(dst[r : r + step, :], t_[:])

            # AllGather buffers, one per t-chunk (column-sliced attn^T).
            cc_in = [dram.tile([OL, TC], BF16, name=f"cc_in{c}") for c in range(NTC)]
            cc_g = [
                dram.tile([TPG * OL, TC], BF16, name=f"cc_g{c}") for c in range(NTC)
            ]
            HH = OL // 2  # 512 rows = 4 heads
            cc_in3 = [dram.tile([HH, TC], BF16, name=f"cc_in3{i}") for i in range(2)]
            cc_g3 = [dram.tile([TPG * HH, TC], BF16, name=f"cc_g3{i}") for i in range(2)]

            woT = wpool.tile([P, ND, OL], BF16, tag="bigw", name="woT")

            def emit_outproj3():
                c = NTC - 1
                gT = slabp.tile([P, ND, TC], BF16, tag="slab", name="gT3")
                ots = [ot for ot in range(ND) if ot % HL < 4] + [
                    ot for ot in range(ND) if ot % HL >= 4
                ]
                for ot in ots:
                    r, lh = divmod(ot, HL)
                    half, lh2 = divmod(lh, 4)
                    nc.sync.dma_start(
                        gT[:, ot, :],
                        cc_g3[half][r * HH + lh2 * P : r * HH + (lh2 + 1) * P, :],
                    )
                for tt in range(TC // P):
                    for dc in range(OL // TC):
                        ps = psump.tile(
                            [P, TC], F32, tag="tr", bufs=2, name=f"pso3_{tt}_{dc}"
                        )
                        for i, ot in enumerate(ots):
                            nc.tensor.matmul(
                                ps[:],
                                gT[:, ot, tt * P : (tt + 1) * P],
                                woT[:, ot, dc * TC : (dc + 1) * TC],
                                start=(i == 0),
                                stop=(i == ND - 1),
                            )
                        osb = tmpp.tile([P, TC], F32, tag="osb", bufs=2, name=f"osb3{tt}{dc}")
                        nc.vector.tensor_tensor(
                            osb[:],
                            ps[:],
                            bo_bc[:, dc * TC : (dc + 1) * TC],
                            mybir.AluOpType.add,
                        )
                        nc.sync.dma_start(
                            out_ext[
                                c * TC + tt * P : c * TC + (tt + 1) * P,
                                dc * TC : (dc + 1) * TC,
                            ],
                            osb[:],
                        )

            def emit_outproj(c):
                gT = slabp.tile([P, ND, TC], BF16, tag="slab", name=f"gT{c}")
                for ot in range(ND):
                    r, lh = divmod(ot, HL)
                    nc.sync.dma_start(
                        gT[:, ot, :],
                        cc_g[c][r * OL + lh * P : r * OL + (lh + 1) * P, :],
                    )
                for tt in range(TC // P):
                    for dc in range(OL // TC):
                        ps = psump.tile(
                            [P, TC], F32, tag="tr", bufs=2, name=f"pso{c}_{tt}_{dc}"
                        )
                        for ot in range(ND):
                            nc.tensor.matmul(
                                ps[:],
                                gT[:, ot, tt * P : (tt + 1) * P],
                                woT[:, ot, dc * TC : (dc + 1) * TC],
                                start=(ot == 0),
                                stop=(ot == ND - 1),
                            )
                        osb = tmpp.tile([P, TC], F32, tag="osb", bufs=2, name=f"osb{c}{tt}{dc}")
                        nc.vector.tensor_tensor(
                            osb[:],
                            ps[:],
                            bo_bc[:, dc * TC : (dc + 1) * TC],
                            mybir.AluOpType.add,
                        )
                        nc.sync.dma_start(
                            out_ext[
                                c * TC + tt * P : c * TC + (tt + 1) * P,
                                dc * TC : (dc + 1) * TC,
                            ],
                            osb[:],
                        )

            # ------------- per t-chunk: QKV proj + attention ------------------
            for c in range(NTC):
                if c == 0:
                    # On-chip transpose: x rows 0..511 (critical path).
                    xT = slabp.tile([P, ND, TC], BF16, tag="slab", name="xT0")
                    for tb in range(TC // P):
                        natx = natp.tile([P, D], BF16, tag="nat", name=f"natx{tb}")
                        nc.gpsimd.dma_start(natx[:], x_ext[tb * P : (tb + 1) * P, :])
                        for g in range(0, ND, 4):
                            pst = psump.tile(
                                [P, 4 * P], BF16, tag="attn", bufs=4, name=f"ptrx{tb}{g}"
                            )
                            for j in range(4):
                                nc.tensor.transpose(
                                    pst[:, j * P : (j + 1) * P],
                                    natx[:, (g + j) * P : (g + j + 1) * P],
                                    ident[:],
                                )
                            nc.vector.tensor_copy(
                                xT[:, g : g + 4, tb * P : (tb + 1) * P],
                                pst[:].rearrange("p (g t) -> p g t", g=4),
                            )
                    load_T_onchip(wq_ext, OL, wqT, 0, "wq")
                    load_T_onchip(wk_ext, HD, wkT, 0, "wk")
                    load_T_onchip(wv_ext, HD, wvT, 0, "wv")
                else:
                    xT = slabp.tile([P, ND, TC], BF16, tag="slab", name=f"xT{c}")
                    nc.sync.dma_start(
                        xT[:], x_bf[c * TC : (c + 1) * TC, :], transpose=True
                    )

                qTc = qtcp.tile([P, HL, TC], BF16, tag="qTc", name=f"qTc{c}")

                for ot in range(HL + 2):  # 0..7 = q heads, 8 = k, 9 = v
                    ps = psump.tile(
                        [P, TC], F32, tag="mm512", bufs=2, name=f"psqkv{c}_{ot}"
                    )
                    for dt in range(ND):
                        if ot < HL:
                            lhsT = wqT[:, dt, ot * P : (ot + 1) * P]
                        elif ot == HL:
                            lhsT = wkT[:, dt, :]
                        else:
                            lhsT = wvT[:, dt, :]
                        nc.tensor.matmul(
                            ps[:],
                            lhsT,
                            xT[:, dt, :],
                            start=(dt == 0),
                            stop=(dt == ND - 1),
                        )
                    if ot < HL:
                        nc.vector.tensor_scalar_add(
                            qTc[:, ot, :], ps[:], bq_sb[:, ot : ot + 1]
                        )
                    elif ot == HL:
                        nc.vector.tensor_scalar_add(
                            kT[:, c * TC : (c + 1) * TC], ps[:], bk_sb[:]
                        )
                    else:
                        vt = tmpp.tile([P, TC], BF16, tag="vt", bufs=1, name=f"vt{c}")
                        nc.vector.tensor_scalar_add(vt[:], ps[:], bv_sb[:])
                        pstv = psump.tile(
                            [P, 4 * P], BF16, tag="attn", bufs=4, name=f"pstv{c}"
                        )
                        for j in range(TC // P):
                            nc.tensor.transpose(
                                pstv[:, j * P : (j + 1) * P],
                                vt[:, j * P : (j + 1) * P],
                                ident[:],
                            )
                        nc.vector.tensor_copy(
                            vaug[:, c * (TC // P) : (c + 1) * (TC // P), 0:HD],
                            pstv[:].rearrange("p (g t) -> p g t", g=4),
                        )

                # After chunk-0 compute is queued, enqueue the deferred casts:
                # x chunks 1-3 first (needed soonest), then wo.
                if c == 0:
                    cast_to_scratch(x_ext, x_bf, TC, T, "x")
                    cast_to_scratch(wo_ext, wo_bf, 0, OL, "wo")


                if c == NTC - 1:
                    nc.sync.dma_start(
                        woT[:, :, 0 : OL // 2], wo_bf[0 : OL // 2, :], transpose=True
                    )
                    nc.sync.dma_start(
                        woT[:, :, OL // 2 : OL], wo_bf[OL // 2 : OL, :], transpose=True
                    )

                # Attention for all local heads on this q-chunk.
                attnT_c = atcp.tile([P, HL, TC], BF16, tag="atc", name=f"attnT{c}")
                nkt = (c + 1) * (TC // P)  # causal: k-tiles 0..nkt-1
                for h in range(HL):
                    att_ps = [
                        psump.tile(
                            [P, HD + 1], F32, tag="attn", bufs=4, name=f"att{c}_{h}_{qt}"
                        )
                        for qt in range(TC // P)
                    ]
                    for kt in range(nkt):
                        # Band tiles only need columns tq >= (kt-4c)*128; after
                        # trimming, the causal pattern is always the diagonal.
                        off = (kt - (nkt - 4)) * P if kt >= nkt - 4 else 0
                        ne = TC - off
                        ps_s = psump.tile(
                            [P, TC], F32, tag="mm512", bufs=2, name=f"pss{c}_{h}_{kt}"
                        )
                        nc.tensor.matmul(
                            ps_s[:, :ne],
                            kT[:, kt * P : (kt + 1) * P],
                            qTc[:, h, off:TC],
                            start=True,
                            stop=True,
                        )
                        es = esbp.tile([P, TC], BF16, tag="esb", name=f"es{c}_{h}_{kt}")
                        nc.scalar.activation(
                            es[:, :ne],
                            ps_s[:, :ne],
                            mybir.ActivationFunctionType.Exp,
                            scale=SCALE,
                        )
                        if kt >= nkt - 4:
                            # Zero weights where k > q (pure diagonal after trim).
                            nc.vector.tensor_tensor(
                                es[:, :ne], es[:, :ne], mask0[:, :ne],
                                mybir.AluOpType.mult,
                            )
                        for qt in range(TC // P):
                            tqi = c * (TC // P) + qt
                            if kt > tqi:
                                continue
                            nc.tensor.matmul(
                                att_ps[qt][:],
                                es[:, qt * P - off : (qt + 1) * P - off],
                                vaug[:, kt, :],
                                start=(kt == 0),
                                stop=(kt == tqi),
                            )
                    psta = psump.tile(
                        [P, 4 * P], BF16, tag="attn", bufs=4, name=f"psta{c}{h}"
                    )
                    for qt in range(TC // P):
                        recip = tmpp.tile([P, 1], F32, tag="recip", bufs=2, name=f"rc{c}{h}{qt}")
                        nc.vector.reciprocal(recip[:], att_ps[qt][:, HD : HD + 1])
                        attn_sb = tmpp.tile(
                            [P, P], BF16, tag="attn_sb", bufs=2, name=f"asb{c}{h}{qt}"
                        )
                        nc.vector.tensor_scalar_mul(
                            attn_sb[:], att_ps[qt][:, 0:HD], recip[:]
                        )
                        nc.tensor.transpose(
                            psta[:, qt * P : (qt + 1) * P], attn_sb[:], ident[:]
                        )
                    nc.vector.tensor_copy(attnT_c[:, h, :], psta[:])
                    if c == NTC - 1 and h == 3:
                        nc.sync.dma_start(
                            cc_in3[0][:, :].rearrange("(h p) t -> p h t", p=P),
                            attnT_c[:, 0:4, :],
                        )
                        nc.gpsimd.collective_compute(
                            "AllGather",
                            mybir.AluOpType.bypass,
                            replica_groups=[[0, 1, 2, 3], [4, 5, 6, 7]],
                            ins=[cc_in3[0][:, :].opt()],
                            outs=[cc_g3[0][:, :].opt()],
                        )
                        emit_outproj(0)
                    if c == NTC - 1 and h == 5:
                        emit_outproj(1)
                    if c == NTC - 1 and h == 7:
                        emit_outproj(2)

                # Ship this chunk's attn^T and AllGather it within the group.
                if c < NTC - 1:
                    nc.sync.dma_start(
                        cc_in[c][:, :].rearrange("(h p) t -> p h t", p=P), attnT_c[:]
                    )
                    nc.gpsimd.collective_compute(
                        "AllGather",
                        mybir.AluOpType.bypass,
                        replica_groups=[[0, 1, 2, 3], [4, 5, 6, 7]],
                        ins=[cc_in[c][:, :].opt()],
                        outs=[cc_g[c][:, :].opt()],
                    )
                else:
                    nc.sync.dma_start(
                        cc_in3[1][:, :].rearrange("(h p) t -> p h t", p=P),
                        attnT_c[:, 4:HL, :],
                    )
                    nc.gpsimd.collective_compute(
                        "AllGather",
                        mybir.AluOpType.bypass,
                        replica_groups=[[0, 1, 2, 3], [4, 5, 6, 7]],
                        ins=[cc_in3[1][:, :].opt()],
                        outs=[cc_g3[1][:, :].opt()],
                    )
                    emit_outproj3()

            # (output projection is emitted interleaved, see emit_outproj above)

    nc.compile()
    return nc


def kernel(x, wq_w, wq_b, wk_w, wk_b, wv_w, wv_b, wo_w, wo_b):
    global LAST_RESULT
    if "nc" not in _CACHE:
        _CACHE["nc"] = build_nc()
    nc = _CACHE["nc"]

    f32 = np.float32
    x = np.asarray(x, f32)
    in_maps = []
    for c in range(NCORES):
        b, g = divmod(c, TPG)
        sl = slice(OL * g, OL * (g + 1))
        in_maps.append(
            {
                "x": np.ascontiguousarray(x[b]),
                "wq": np.ascontiguousarray(np.asarray(wq_w, f32)[sl]),
                "bq": np.ascontiguousarray(np.asarray(wq_b, f32)[sl]),
                "wk": np.ascontiguousarray(np.asarray(wk_w, f32)),
                "bk": np.ascontiguousarray(np.asarray(wk_b, f32)),
                "wv": np.ascontiguousarray(np.asarray(wv_w, f32)),
                "bv": np.ascontiguousarray(np.asarray(wv_b, f32)),
                "wo": np.ascontiguousarray(np.asarray(wo_w, f32)[sl]),
                "bo": np.ascontiguousarray(np.asarray(wo_b, f32)[sl]),
            }
        )

    res = run_bass_kernel_spmd(nc, in_maps, core_ids=list(range(NCORES)))
    LAST_RESULT = res

    out = np.empty((B, T, D), dtype=f32)
    for c in range(NCORES):
        b, g = divmod(c, TPG)
        out[b, :, OL * g : OL * (g + 1)] = res.results[c]["out"]
    return out
